# revision 1
# baseline (speedup 1.0000x reference)
"""Trainium2 Bass kernel for nn_BeBertEmbedding (self-contained).

Math: the reference's semantic_embed(ids, W, b, pad=0) is
    where(ids==0, take(W.T, ids) + b, zeros)
so the only table row that survives is W[:, 0], and the whole module is
    out[b,s,:] = pe[s,:] + (ids[b,s]==0)*(W_tok[:,0]+b_tok)
                         + (seg[b,s]==0)*(W_seg[:,0]+b_seg)

Sharding: sequence-parallel across 8 cores (256 positions/core, all 16
batches; each core writes a disjoint [16, 256, 768] slice, no collectives).

Per core the device program is raw Bass (no Tile — avoids the kernel-tail
drain/barrier):
  * one packed const tensor: [c_seg | seg-masks | (c_tok | tok-masks) |
    pe tile0 | pe tile1], loaded in two DMAs (the first covers everything
    the t=0 tiles need, so compute starts before pe tile1 lands),
  * 32 output tiles [128 tokens, 768]: one fused DVE scalar_tensor_tensor
    (c_seg * m2[p]) + pe each (plus a second op for the rare tiles with a
    zero token id), through NSLOT SBUF slots,
  * out-DMAs issued alternately from the SP and ACT sequencers — TRN2's two
    physical HW-DGE rings — keeping the stream HBM-write-bandwidth-bound,
  * per-slot semaphores (DMAs on different HW queues complete out of order,
    a single counting sem cannot tell WHICH slot freed).

Cost-model estimate 33.2us/core; streams in isolation: DVE ~29.4us, HWDGE
writes 24.0us (model; real HBM is ~358GB/s per core SHARED by reads and
writes, so the real floor is (12MB out + 0.8MB in)/358GB/s ~= 36us — this
kernel sits on that roofline).  Rejected variants kept behind env flags:
BASS_KERNEL_POOL_SPLIT (walrus: no TensorScalarPtr on Pool),
BASS_KERNEL_DEV_BCAST (on-device c_seg broadcast saves 0.4MB HBM read but
the cvec DMA + gpsimd ucode library reload + broadcast chain costs more
head than it saves).  An indirect-DMA scatter variant (2 DVE ops + 2 SWDGE
scatters, see sim_scatter.py in the dev tree) sims at 10.6us but that is a
cost-model artifact — it does not charge SWDGE data movement; real SWDGE
queue->engine spread is unknown, so it is not shipped.
"""

import contextlib
import os
import sys

import numpy as np

try:
    from concourse import bacc, bass, mybir, tile
    from concourse.bass_utils import run_bass_kernel_spmd
except ImportError:
    for _p in ("/opt/trn_rl_repo", "/root/.axon_site/_ro/trn_rl_repo"):
        if os.path.isdir(_p) and _p not in sys.path:
            sys.path.insert(0, _p)
            break
    from concourse import bacc, bass, mybir, tile
    from concourse.bass_utils import run_bass_kernel_spmd

N_CORES = 8
B, SEQ, D = 16, 2048, 768
S_SH = SEQ // N_CORES        # 256 sequence positions per core
P = 128                      # partitions
T_TILES = S_SH // P          # 2 seq tiles per core
J = B * T_TILES              # 32 output tiles per core
NSLOT = 16                   # SBUF output slots

_F32 = mybir.dt.float32

_prog_cache: dict = {}
LAST_RESULTS = None          # BassKernelResults of the most recent run


# ---------------------------------------------------------------- raw bass —

def _layout(tok_cols, dev_bcast=False):
    """Column offsets inside the packed per-core const tensor [128, C].
    With dev_bcast the c-vectors are NOT in this tensor (they arrive as a
    [1, D]-per-vector tensor and are partition-broadcast on device)."""
    if dev_bcast:
        M2_OFF = 0
        off = M2_OFF + J
        if tok_cols:
            M1_OFF = off
            off = M1_OFF + J
        else:
            M1_OFF = None
        CSEG_OFF = CTOK_OFF = None
        PE_OFF = off
        C = PE_OFF + T_TILES * D
        A_COLS = PE_OFF + D
        return CSEG_OFF, M2_OFF, CTOK_OFF, M1_OFF, PE_OFF, C, A_COLS
    CSEG_OFF = 0
    M2_OFF = CSEG_OFF + D
    off = M2_OFF + J
    if tok_cols:
        CTOK_OFF = off
        M1_OFF = CTOK_OFF + D
        off = M1_OFF + J
    else:
        CTOK_OFF = M1_OFF = None
    PE_OFF = off
    C = PE_OFF + T_TILES * D
    A_COLS = PE_OFF + D          # DMA-A: everything + pe tile 0
    return CSEG_OFF, M2_OFF, CTOK_OFF, M1_OFF, PE_OFF, C, A_COLS


def _order(i):
    """Compute-order index i -> output tile j: all t=0 tiles first (they
    only need the first const DMA), then t=1 tiles."""
    return 2 * i if i < J // 2 else 2 * (i - J // 2) + 1


def _build_raw(
    tok_cols: frozenset, pool_split: bool = False, dev_bcast: bool = False
) -> "bass.Bass":
    CSEG_OFF, M2_OFF, CTOK_OFF, M1_OFF, PE_OFF, C, A_COLS = _layout(
        tok_cols, dev_bcast
    )
    assert not (pool_split and dev_bcast)
    NV = 2 if tok_cols else 1    # number of broadcast c-vectors
    nc = bass.Bass("TRN2")
    const_d = nc.dram_tensor("consts", [P, C], _F32, kind="ExternalInput")
    if dev_bcast:
        cvec_d = nc.dram_tensor("cvec", [1, NV * D], _F32, kind="ExternalInput")
    out_d = nc.dram_tensor("out", [B * S_SH, D], _F32, kind="ExternalOutput")
    mult, add = mybir.AluOpType.mult, mybir.AluOpType.add

    # Which compute order-indices run on GPSIMD (Pool) instead of DVE.
    # i%4==3 keeps slot ownership disjoint (slots 3,7 are pool-exclusive, so
    # same-slot WAR chains never cross engines).  Tok tiles stay on DVE.
    if pool_split:
        on_pool = [i % 4 == 3 and _order(i) not in tok_cols for i in range(J)]
    else:
        on_pool = [False] * J
    # per-engine completion counts up to and including index i
    dve_cnt, pool_cnt, dc, pc = [], [], 0, 0
    for i in range(J):
        if on_pool[i]:
            pc += 1
        else:
            dc += 1
        dve_cnt.append(dc)
        pool_cnt.append(pc)

    with contextlib.ExitStack() as stack:
        c_t = stack.enter_context(nc.sbuf_tensor([P, C], _F32))
        if dev_bcast:
            cw = stack.enter_context(nc.sbuf_tensor([P, NV * D], _F32))
            cv_sem = stack.enter_context(nc.semaphore("cv_sem"))
            bc_sem = stack.enter_context(nc.semaphore("bc_sem"))
        obuf = stack.enter_context(nc.sbuf_tensor([P, NSLOT * D], _F32))
        a_sem = stack.enter_context(nc.semaphore("a_sem"))
        b_sem = stack.enter_context(nc.semaphore("b_sem"))
        v_sem = stack.enter_context(nc.semaphore("v_sem"))
        p_sem = stack.enter_context(nc.semaphore("p_sem"))
        s_sems = [
            stack.enter_context(nc.semaphore(f"slot_sem{k}")) for k in range(NSLOT)
        ]
        # DVE's pipeline is deep: the tok tile's second STT reads the first's
        # output on the same engine and needs an explicit retire guard.
        t_sem = stack.enter_context(nc.semaphore("tok_sem"))
        block = stack.enter_context(nc.Block())

        def issue_out_dmas(eng, parity):
            for i in range(J):
                if i % 2 != parity:
                    continue
                j = _order(i)
                s = i % NSLOT
                done = (p_sem, pool_cnt[i]) if on_pool[i] else (v_sem, dve_cnt[i])
                # wait attached inline: TRN2 allows exactly one wait per
                # instruction, and this saves a sequencer dispatch slot
                eng.dma_start(
                    out_d[j * P:(j + 1) * P, :], obuf[:, s * D:(s + 1) * D]
                )._wait_ge(*done).then_inc(s_sems[s], 16)

        # Head: the first compute op needs masks + cseg + pe tile 0.  Those
        # load as THREE parallel-ish DMAs: [masks|cseg] then pe0-low-half on
        # the SP ring, pe0-high-half on the ACT ring — halving the largest
        # serial transfer in front of the first op.  pe1 (only needed from
        # i=J/2) follows on SP.
        PE0 = PE_OFF
        PE0M = PE_OFF + D // 2
        @block.sync
        def _(sync):
            if dev_bcast:
                sync.dma_start(cw[0:1, :], cvec_d[:]).then_inc(cv_sem, 16)
            sync.dma_start(c_t[:, :PE0], const_d[:, :PE0]).then_inc(a_sem, 16)
            sync.dma_start(c_t[:, PE0:PE0M], const_d[:, PE0:PE0M]).then_inc(a_sem, 16)
            sync.dma_start(c_t[:, A_COLS:], const_d[:, A_COLS:]).then_inc(b_sem, 16)
            issue_out_dmas(sync, 0)

        @block.scalar
        def _(scalar):
            scalar.dma_start(
                c_t[:, PE0M:A_COLS], const_d[:, PE0M:A_COLS]
            ).then_inc(a_sem, 16)
            issue_out_dmas(scalar, 1)

        if dev_bcast:
            @block.gpsimd
            def _(gpsimd):
                from concourse import library_config
                # PartitionBroadcast lives in the attnmlp/attn/mlp/proxy
                # gpsimd ucode libraries, not the default one
                nc.gpsimd.load_library(library_config.attnmlp)
                gpsimd.wait_ge(cv_sem, 16)
                for v in range(NV):
                    nc.gpsimd.partition_broadcast(
                        cw[:, v * D:(v + 1) * D], cw[0:1, v * D:(v + 1) * D]
                    ).then_inc(bc_sem, 1)

        def compute(eng_handle, eng_obj, my_flag, done_sem):
            if dev_bcast:
                cseg = cw[:, 0:D]
            else:
                cseg = c_t[:, CSEG_OFF:CSEG_OFF + D]
            n_tok = 0
            waited_b = False
            eng_handle.wait_ge(a_sem, 48)   # all three pieces of the A-load
            if dev_bcast:
                eng_handle.wait_ge(bc_sem, NV)
            for i in range(J):
                if on_pool[i] != my_flag:
                    continue
                j = _order(i)
                t = j % T_TILES
                s = i % NSLOT
                if t == 1 and not waited_b:
                    eng_handle.wait_ge(b_sem, 16)
                    waited_b = True
                # slot-reuse wait attached inline on the STT (one wait max
                # per instruction; saves a sequencer dispatch slot)
                slot_wait = (
                    (s_sems[s], 16 * (i // NSLOT)) if i >= NSLOT else None
                )
                o_sl = obuf[:, s * D:(s + 1) * D]
                pe_sl = c_t[:, PE_OFF + t * D:PE_OFF + (t + 1) * D]
                m2_col = c_t[:, M2_OFF + j:M2_OFF + j + 1]
                if j in tok_cols:
                    m1_col = c_t[:, M1_OFF + j:M1_OFF + j + 1]
                    if dev_bcast:
                        ctok = cw[:, D:2 * D]
                    else:
                        ctok = c_t[:, CTOK_OFF:CTOK_OFF + D]
                    # acc = m1*c_tok + pe, then m2*c_seg + acc: the same fp
                    # add order as the reference's (tok + pe) + seg.
                    eng_obj.scalar_tensor_tensor(
                        o_sl, ctok, m1_col, pe_sl, op0=mult, op1=add,
                    )._maybe_wait_ge(slot_wait).then_inc(t_sem, 1)
                    n_tok += 1
                    eng_obj.scalar_tensor_tensor(
                        o_sl, cseg, m2_col, o_sl, op0=mult, op1=add,
                    )._wait_ge(t_sem, n_tok).then_inc(done_sem, 1)
                else:
                    eng_obj.scalar_tensor_tensor(
                        o_sl, cseg, m2_col, pe_sl, op0=mult, op1=add,
                    )._maybe_wait_ge(slot_wait).then_inc(done_sem, 1)

        @block.vector
        def _(vector):
            compute(vector, nc.vector, False, v_sem)

        if pool_split:
            @block.gpsimd
            def _(gpsimd):
                compute(gpsimd, nc.gpsimd, True, p_sem)

    nc.finalize()
    return nc


def _prepare_raw(inputs: dict, dev_bcast: bool = False):
    ids = np.asarray(inputs["input_ids"])
    seg = np.asarray(inputs["segment_label"])
    W_tok = np.asarray(inputs["W_tok"], dtype=np.float32)
    b_tok = np.asarray(inputs["b_tok"], dtype=np.float32)
    W_seg = np.asarray(inputs["W_seg"], dtype=np.float32)
    b_seg = np.asarray(inputs["b_seg"], dtype=np.float32)
    pe = np.asarray(inputs["pe"], dtype=np.float32).reshape(SEQ, D)

    c_tok = (W_tok[:, 0] + b_tok).astype(np.float32)
    c_seg = (W_seg[:, 0] + b_seg).astype(np.float32)
    m1_full = (ids == 0).astype(np.float32)
    m2_full = (seg == 0).astype(np.float32)

    per_core = []
    tok_cols = set()
    for c in range(N_CORES):
        sl = slice(c * S_SH, (c + 1) * S_SH)
        # [B, S_SH] -> [P, J] with column j = b*T_TILES + t, partition p
        m1 = m1_full[:, sl].reshape(B, T_TILES, P).transpose(2, 0, 1).reshape(P, J)
        m2 = m2_full[:, sl].reshape(B, T_TILES, P).transpose(2, 0, 1).reshape(P, J)
        pe_sl = pe[sl].reshape(T_TILES, P, D).transpose(1, 0, 2).reshape(P, T_TILES * D)
        tok_cols.update(np.nonzero(m1.any(axis=0))[0].tolist())
        per_core.append((pe_sl, m1, m2))

    tok_cols = frozenset(tok_cols)
    CSEG_OFF, M2_OFF, CTOK_OFF, M1_OFF, PE_OFF, C, _ = _layout(tok_cols, dev_bcast)
    NV = 2 if tok_cols else 1
    cvec = None
    if dev_bcast:
        cvec = np.empty((1, NV * D), dtype=np.float32)
        cvec[0, :D] = c_seg
        if tok_cols:
            cvec[0, D:] = c_tok
    in_maps = []
    for pe_sl, m1, m2 in per_core:
        consts = np.empty((P, C), dtype=np.float32)
        consts[:, M2_OFF:M2_OFF + J] = m2
        if tok_cols:
            consts[:, M1_OFF:M1_OFF + J] = m1
        if not dev_bcast:
            consts[:, CSEG_OFF:CSEG_OFF + D] = c_seg
            if tok_cols:
                consts[:, CTOK_OFF:CTOK_OFF + D] = c_tok
        consts[:, PE_OFF:PE_OFF + T_TILES * D] = pe_sl
        m = {"consts": consts}
        if dev_bcast:
            m["cvec"] = cvec
        in_maps.append(m)
    return in_maps, tok_cols


# -------------------------------------------------- tile variant (fallback) —

TPE_OFF = 0
TCSEG_OFF = T_TILES * D
TM2_OFF = TCSEG_OFF + D
TC_COMMON = TM2_OFF + J
TCTOK_OFF = TC_COMMON
TM1_OFF = TCTOK_OFF + D
TC_FULL = TM1_OFF + J


def _build_tile(tok_cols: frozenset) -> "bass.Bass":
    C = TC_FULL if tok_cols else TC_COMMON
    # Bacc (not plain Bass): its compile() splits multi-semaphore waits into
    # event semaphores — TRN2 allows at most one inline wait per instruction.
    nc = bacc.Bacc("TRN2", target_bir_lowering=False)
    const_d = nc.dram_tensor("consts", [P, C], _F32, kind="ExternalInput")
    out_d = nc.dram_tensor("out", [B * S_SH, D], _F32, kind="ExternalOutput")
    mult, add = mybir.AluOpType.mult, mybir.AluOpType.add

    with tile.TileContext(nc) as tc:
        with (
            tc.tile_pool(name="const", bufs=1) as cpool,
            tc.tile_pool(name="outp", bufs=8) as opool,
        ):
            c_t = cpool.tile([P, C], _F32)
            nc.sync.dma_start(c_t[:], const_d[:])
            cseg_t = c_t[:, TCSEG_OFF:TCSEG_OFF + D]
            ctok_t = c_t[:, TCTOK_OFF:TCTOK_OFF + D] if tok_cols else None

            for b in range(B):
                for t in range(T_TILES):
                    j = b * T_TILES + t
                    o = opool.tile([P, D], _F32, tag="out")
                    pe_slice = c_t[:, TPE_OFF + t * D:TPE_OFF + (t + 1) * D]
                    m2_col = c_t[:, TM2_OFF + j:TM2_OFF + j + 1]
                    if j in tok_cols:
                        m1_col = c_t[:, TM1_OFF + j:TM1_OFF + j + 1]
                        nc.vector.scalar_tensor_tensor(
                            o[:], ctok_t, m1_col, pe_slice, op0=mult, op1=add,
                        )
                        nc.vector.scalar_tensor_tensor(
                            o[:], cseg_t, m2_col, o[:], op0=mult, op1=add,
                        )
                    else:
                        nc.vector.scalar_tensor_tensor(
                            o[:], cseg_t, m2_col, pe_slice, op0=mult, op1=add,
                        )
                    nc.sync.dma_start(out_d[j * P:(j + 1) * P, :], o[:])
    nc.finalize()
    return nc


def _prepare_tile(inputs: dict):
    ids = np.asarray(inputs["input_ids"])
    seg = np.asarray(inputs["segment_label"])
    W_tok = np.asarray(inputs["W_tok"], dtype=np.float32)
    b_tok = np.asarray(inputs["b_tok"], dtype=np.float32)
    W_seg = np.asarray(inputs["W_seg"], dtype=np.float32)
    b_seg = np.asarray(inputs["b_seg"], dtype=np.float32)
    pe = np.asarray(inputs["pe"], dtype=np.float32).reshape(SEQ, D)

    c_tok = (W_tok[:, 0] + b_tok).astype(np.float32)
    c_seg = (W_seg[:, 0] + b_seg).astype(np.float32)
    m1_full = (ids == 0).astype(np.float32)
    m2_full = (seg == 0).astype(np.float32)

    per_core = []
    tok_cols = set()
    for c in range(N_CORES):
        sl = slice(c * S_SH, (c + 1) * S_SH)
        m1 = m1_full[:, sl].reshape(B, T_TILES, P).transpose(2, 0, 1).reshape(P, J)
        m2 = m2_full[:, sl].reshape(B, T_TILES, P).transpose(2, 0, 1).reshape(P, J)
        pe_sl = pe[sl].reshape(T_TILES, P, D).transpose(1, 0, 2).reshape(P, T_TILES * D)
        tok_cols.update(np.nonzero(m1.any(axis=0))[0].tolist())
        per_core.append((pe_sl, m1, m2))

    need_tok = bool(tok_cols)
    C = TC_FULL if need_tok else TC_COMMON
    in_maps = []
    for pe_sl, m1, m2 in per_core:
        consts = np.empty((P, C), dtype=np.float32)
        consts[:, TPE_OFF:TPE_OFF + T_TILES * D] = pe_sl
        consts[:, TCSEG_OFF:TCSEG_OFF + D] = c_seg
        consts[:, TM2_OFF:TM2_OFF + J] = m2
        if need_tok:
            consts[:, TCTOK_OFF:TCTOK_OFF + D] = c_tok
            consts[:, TM1_OFF:TM1_OFF + J] = m1
        in_maps.append({"consts": consts})
    return in_maps, frozenset(tok_cols)


# ------------------------------------------------------------------- entry —

def kernel(**inputs) -> np.ndarray:
    global LAST_RESULTS
    impl = os.environ.get("BASS_KERNEL_IMPL", "raw")
    if impl == "raw":
        # NOTE: pool_split compiles in CoreSim but walrus rejects
        # TensorScalarPtr on the Pool engine (NCC_IXCG966) — keep off.
        pool_split = bool(int(os.environ.get("BASS_KERNEL_POOL_SPLIT", "0")))
        dev_bcast = bool(int(os.environ.get("BASS_KERNEL_DEV_BCAST", "0")))
        in_maps, tok_cols = _prepare_raw(inputs, dev_bcast=dev_bcast)
        key = ("raw", pool_split, dev_bcast, tok_cols)
        def builder(tc):
            return _build_raw(tc, pool_split=pool_split, dev_bcast=dev_bcast)
    else:
        in_maps, tok_cols = _prepare_tile(inputs)
        key = ("tile", tok_cols)
        builder = _build_tile
    # SPMD: one program for all cores; the tok op is emitted for any column
    # that needs it on any core (a zero mask column makes it the identity).
    if key not in _prog_cache:
        _prog_cache[key] = builder(tok_cols)
    nc = _prog_cache[key]

    trace = bool(int(os.environ.get("BASS_KERNEL_TRACE", "0")))
    try:
        res = run_bass_kernel_spmd(
            nc, in_maps, list(range(N_CORES)), trace=trace,
            trace_cores=list(range(N_CORES)) if trace else None,
        )
    except ModuleNotFoundError:
        # axon builds without the NTFF profile hook (antenv.axon_hooks)
        # crash when tracing is requested (e.g. BASS_TRACE=1 in the env);
        # degrade to an untraced run rather than failing the kernel.
        os.environ["BASS_NEVER_TRACE"] = "1"
        res = run_bass_kernel_spmd(nc, in_maps, list(range(N_CORES)), trace=False)
    LAST_RESULTS = res

    out = np.empty((B, SEQ, D), dtype=np.float32)
    for c in range(N_CORES):
        out[:, c * S_SH:(c + 1) * S_SH, :] = (
            np.asarray(res.results[c]["out"]).reshape(B, S_SH, D)
        )
    return out



# revision 3
# speedup vs baseline: 1.0210x; 1.0210x over previous
"""Trainium2 Bass kernel for nn_BeBertEmbedding (self-contained).

Math: the reference's semantic_embed(ids, W, b, pad=0) is
    where(ids==0, take(W.T, ids) + b, zeros)
so the only table row that survives is W[:, 0], and the whole module is
    out[b,s,:] = pe[s,:] + (ids[b,s]==0)*(W_tok[:,0]+b_tok)
                         + (seg[b,s]==0)*(W_seg[:,0]+b_seg)

Sharding: sequence-parallel across 8 cores (256 positions/core, all 16
batches; each core writes a disjoint [16, 256, 768] slice, no collectives).

Per core the device program is raw Bass (no Tile — avoids the kernel-tail
drain/barrier):
  * one packed const tensor: [c_seg | seg-masks | (c_tok | tok-masks) |
    pe tile0 | pe tile1], loaded in two DMAs (the first covers everything
    the t=0 tiles need, so compute starts before pe tile1 lands),
  * 32 output tiles [128 tokens, 768]: one fused DVE scalar_tensor_tensor
    (c_seg * m2[p]) + pe each (plus a second op for the rare tiles with a
    zero token id), through NSLOT SBUF slots,
  * out-DMAs issued alternately from the SP and ACT sequencers — TRN2's two
    physical HW-DGE rings — keeping the stream HBM-write-bandwidth-bound,
  * per-slot semaphores (DMAs on different HW queues complete out of order,
    a single counting sem cannot tell WHICH slot freed).

Cost-model estimate 33.2us/core; streams in isolation: DVE ~29.4us, HWDGE
writes 24.0us (model; real HBM is ~358GB/s per core SHARED by reads and
writes, so the real floor is (12MB out + 0.8MB in)/358GB/s ~= 36us — this
kernel sits on that roofline).  Rejected variants kept behind env flags:
BASS_KERNEL_POOL_SPLIT (walrus: no TensorScalarPtr on Pool),
BASS_KERNEL_DEV_BCAST (on-device c_seg broadcast saves 0.4MB HBM read but
the cvec DMA + gpsimd ucode library reload + broadcast chain costs more
head than it saves).  An indirect-DMA scatter variant (2 DVE ops + 2 SWDGE
scatters, see sim_scatter.py in the dev tree) sims at 10.6us but that is a
cost-model artifact — it does not charge SWDGE data movement; real SWDGE
queue->engine spread is unknown, so it is not shipped.
"""

import contextlib
import os
import sys

import numpy as np

try:
    from concourse import bacc, bass, mybir, tile
    from concourse.bass_utils import run_bass_kernel_spmd
except ImportError:
    for _p in ("/opt/trn_rl_repo", "/root/.axon_site/_ro/trn_rl_repo"):
        if os.path.isdir(_p) and _p not in sys.path:
            sys.path.insert(0, _p)
            break
    from concourse import bacc, bass, mybir, tile
    from concourse.bass_utils import run_bass_kernel_spmd

N_CORES = 8
B, SEQ, D = 16, 2048, 768
S_SH = SEQ // N_CORES        # 256 sequence positions per core
P = 128                      # partitions
T_TILES = S_SH // P          # 2 seq tiles per core
J = B * T_TILES              # 32 output tiles per core
NSLOT = 16                   # SBUF output slots

_F32 = mybir.dt.float32
_F16 = mybir.dt.float16

# On-device dtype for everything the kernel touches (consts, compute, out).
# fp16 halves HBM traffic and doubles DVE throughput; the harness tolerance
# is rel_err < 2e-2 and fp16 keeps us at ~1e-3 (values are pe in [-1,1]
# plus ~0.08-max corrections, well inside fp16's 2^-11 relative precision).
# The host upcasts the returned fp16 tiles to the reference's float32.
_DEV_DT = _F16
_DEV_NP = np.float16

_prog_cache: dict = {}
LAST_RESULTS = None          # BassKernelResults of the most recent run


# ---------------------------------------------------------------- raw bass —

def _layout(tok_cols, dev_bcast=False):
    """Column offsets inside the packed per-core const tensor [128, C].
    With dev_bcast the c-vectors are NOT in this tensor (they arrive as a
    [1, D]-per-vector tensor and are partition-broadcast on device)."""
    if dev_bcast:
        M2_OFF = 0
        off = M2_OFF + J
        if tok_cols:
            M1_OFF = off
            off = M1_OFF + J
        else:
            M1_OFF = None
        CSEG_OFF = CTOK_OFF = None
        PE_OFF = off
        C = PE_OFF + T_TILES * D
        A_COLS = PE_OFF + D
        return CSEG_OFF, M2_OFF, CTOK_OFF, M1_OFF, PE_OFF, C, A_COLS
    CSEG_OFF = 0
    M2_OFF = CSEG_OFF + D
    off = M2_OFF + J
    if tok_cols:
        CTOK_OFF = off
        M1_OFF = CTOK_OFF + D
        off = M1_OFF + J
    else:
        CTOK_OFF = M1_OFF = None
    PE_OFF = off
    C = PE_OFF + T_TILES * D
    A_COLS = PE_OFF + D          # DMA-A: everything + pe tile 0
    return CSEG_OFF, M2_OFF, CTOK_OFF, M1_OFF, PE_OFF, C, A_COLS


def _order(i):
    """Compute-order index i -> output tile j: all t=0 tiles first (they
    only need the first const DMA), then t=1 tiles."""
    return 2 * i if i < J // 2 else 2 * (i - J // 2) + 1


def _build_raw(
    tok_cols: frozenset, pool_split: bool = False, dev_bcast: bool = False
) -> "bass.Bass":
    CSEG_OFF, M2_OFF, CTOK_OFF, M1_OFF, PE_OFF, C, A_COLS = _layout(
        tok_cols, dev_bcast
    )
    assert not (pool_split and dev_bcast)
    NV = 2 if tok_cols else 1    # number of broadcast c-vectors
    nc = bass.Bass("TRN2")
    const_d = nc.dram_tensor("consts", [P, C], _DEV_DT, kind="ExternalInput")
    if dev_bcast:
        cvec_d = nc.dram_tensor("cvec", [1, NV * D], _DEV_DT, kind="ExternalInput")
    out_d = nc.dram_tensor("out", [B * S_SH, D], _DEV_DT, kind="ExternalOutput")
    mult, add = mybir.AluOpType.mult, mybir.AluOpType.add

    # Which compute order-indices run on GPSIMD (Pool) instead of DVE.
    # i%4==3 keeps slot ownership disjoint (slots 3,7 are pool-exclusive, so
    # same-slot WAR chains never cross engines).  Tok tiles stay on DVE.
    if pool_split:
        on_pool = [i % 4 == 3 and _order(i) not in tok_cols for i in range(J)]
    else:
        on_pool = [False] * J
    # per-engine completion counts up to and including index i
    dve_cnt, pool_cnt, dc, pc = [], [], 0, 0
    for i in range(J):
        if on_pool[i]:
            pc += 1
        else:
            dc += 1
        dve_cnt.append(dc)
        pool_cnt.append(pc)

    with contextlib.ExitStack() as stack:
        c_t = stack.enter_context(nc.sbuf_tensor([P, C], _DEV_DT))
        if dev_bcast:
            cw = stack.enter_context(nc.sbuf_tensor([P, NV * D], _DEV_DT))
            cv_sem = stack.enter_context(nc.semaphore("cv_sem"))
            bc_sem = stack.enter_context(nc.semaphore("bc_sem"))
        obuf = stack.enter_context(nc.sbuf_tensor([P, NSLOT * D], _DEV_DT))
        a_sem = stack.enter_context(nc.semaphore("a_sem"))
        b_sem = stack.enter_context(nc.semaphore("b_sem"))
        v_sem = stack.enter_context(nc.semaphore("v_sem"))
        p_sem = stack.enter_context(nc.semaphore("p_sem"))
        s_sems = [
            stack.enter_context(nc.semaphore(f"slot_sem{k}")) for k in range(NSLOT)
        ]
        # DVE's pipeline is deep: the tok tile's second STT reads the first's
        # output on the same engine and needs an explicit retire guard.
        t_sem = stack.enter_context(nc.semaphore("tok_sem"))
        block = stack.enter_context(nc.Block())

        def issue_out_dmas(eng, parity):
            for i in range(J):
                if i % 2 != parity:
                    continue
                j = _order(i)
                s = i % NSLOT
                done = (p_sem, pool_cnt[i]) if on_pool[i] else (v_sem, dve_cnt[i])
                # wait attached inline: TRN2 allows exactly one wait per
                # instruction, and this saves a sequencer dispatch slot
                eng.dma_start(
                    out_d[j * P:(j + 1) * P, :], obuf[:, s * D:(s + 1) * D]
                )._wait_ge(*done).then_inc(s_sems[s], 16)

        # Head: the first compute op needs masks + cseg + pe tile 0.  Those
        # load as THREE parallel-ish DMAs: [masks|cseg] then pe0-low-half on
        # the SP ring, pe0-high-half on the ACT ring — halving the largest
        # serial transfer in front of the first op.  pe1 (only needed from
        # i=J/2) follows on SP.
        PE0 = PE_OFF
        PE0M = PE_OFF + D // 2
        @block.sync
        def _(sync):
            if dev_bcast:
                sync.dma_start(cw[0:1, :], cvec_d[:]).then_inc(cv_sem, 16)
            sync.dma_start(c_t[:, :PE0], const_d[:, :PE0]).then_inc(a_sem, 16)
            sync.dma_start(c_t[:, PE0:PE0M], const_d[:, PE0:PE0M]).then_inc(a_sem, 16)
            sync.dma_start(c_t[:, A_COLS:], const_d[:, A_COLS:]).then_inc(b_sem, 16)
            issue_out_dmas(sync, 0)

        @block.scalar
        def _(scalar):
            scalar.dma_start(
                c_t[:, PE0M:A_COLS], const_d[:, PE0M:A_COLS]
            ).then_inc(a_sem, 16)
            issue_out_dmas(scalar, 1)

        if dev_bcast:
            @block.gpsimd
            def _(gpsimd):
                from concourse import library_config
                # PartitionBroadcast lives in the attnmlp/attn/mlp/proxy
                # gpsimd ucode libraries, not the default one
                nc.gpsimd.load_library(library_config.attnmlp)
                gpsimd.wait_ge(cv_sem, 16)
                for v in range(NV):
                    nc.gpsimd.partition_broadcast(
                        cw[:, v * D:(v + 1) * D], cw[0:1, v * D:(v + 1) * D]
                    ).then_inc(bc_sem, 1)

        def compute(eng_handle, eng_obj, my_flag, done_sem):
            if dev_bcast:
                cseg = cw[:, 0:D]
            else:
                cseg = c_t[:, CSEG_OFF:CSEG_OFF + D]
            n_tok = 0
            waited_b = False
            eng_handle.wait_ge(a_sem, 48)   # all three pieces of the A-load
            if dev_bcast:
                eng_handle.wait_ge(bc_sem, NV)
            for i in range(J):
                if on_pool[i] != my_flag:
                    continue
                j = _order(i)
                t = j % T_TILES
                s = i % NSLOT
                if t == 1 and not waited_b:
                    eng_handle.wait_ge(b_sem, 16)
                    waited_b = True
                # slot-reuse wait attached inline on the STT (one wait max
                # per instruction; saves a sequencer dispatch slot)
                slot_wait = (
                    (s_sems[s], 16 * (i // NSLOT)) if i >= NSLOT else None
                )
                o_sl = obuf[:, s * D:(s + 1) * D]
                pe_sl = c_t[:, PE_OFF + t * D:PE_OFF + (t + 1) * D]
                m2_col = c_t[:, M2_OFF + j:M2_OFF + j + 1]
                if j in tok_cols:
                    m1_col = c_t[:, M1_OFF + j:M1_OFF + j + 1]
                    if dev_bcast:
                        ctok = cw[:, D:2 * D]
                    else:
                        ctok = c_t[:, CTOK_OFF:CTOK_OFF + D]
                    # acc = m1*c_tok + pe, then m2*c_seg + acc: the same fp
                    # add order as the reference's (tok + pe) + seg.
                    eng_obj.scalar_tensor_tensor(
                        o_sl, ctok, m1_col, pe_sl, op0=mult, op1=add,
                    )._maybe_wait_ge(slot_wait).then_inc(t_sem, 1)
                    n_tok += 1
                    eng_obj.scalar_tensor_tensor(
                        o_sl, cseg, m2_col, o_sl, op0=mult, op1=add,
                    )._wait_ge(t_sem, n_tok).then_inc(done_sem, 1)
                else:
                    eng_obj.scalar_tensor_tensor(
                        o_sl, cseg, m2_col, pe_sl, op0=mult, op1=add,
                    )._maybe_wait_ge(slot_wait).then_inc(done_sem, 1)

        @block.vector
        def _(vector):
            compute(vector, nc.vector, False, v_sem)

        if pool_split:
            @block.gpsimd
            def _(gpsimd):
                compute(gpsimd, nc.gpsimd, True, p_sem)

    nc.finalize()
    return nc


def _prepare_raw(inputs: dict, dev_bcast: bool = False):
    ids = np.asarray(inputs["input_ids"])
    seg = np.asarray(inputs["segment_label"])
    W_tok = np.asarray(inputs["W_tok"], dtype=np.float32)
    b_tok = np.asarray(inputs["b_tok"], dtype=np.float32)
    W_seg = np.asarray(inputs["W_seg"], dtype=np.float32)
    b_seg = np.asarray(inputs["b_seg"], dtype=np.float32)
    pe = np.asarray(inputs["pe"], dtype=np.float32).reshape(SEQ, D)

    c_tok = (W_tok[:, 0] + b_tok).astype(_DEV_NP)
    c_seg = (W_seg[:, 0] + b_seg).astype(_DEV_NP)
    m1_full = (ids == 0).astype(_DEV_NP)
    m2_full = (seg == 0).astype(_DEV_NP)

    per_core = []
    tok_cols = set()
    for c in range(N_CORES):
        sl = slice(c * S_SH, (c + 1) * S_SH)
        # [B, S_SH] -> [P, J] with column j = b*T_TILES + t, partition p
        m1 = m1_full[:, sl].reshape(B, T_TILES, P).transpose(2, 0, 1).reshape(P, J)
        m2 = m2_full[:, sl].reshape(B, T_TILES, P).transpose(2, 0, 1).reshape(P, J)
        pe_sl = pe[sl].reshape(T_TILES, P, D).transpose(1, 0, 2).reshape(P, T_TILES * D).astype(_DEV_NP)
        tok_cols.update(np.nonzero(m1.any(axis=0))[0].tolist())
        per_core.append((pe_sl, m1, m2))

    tok_cols = frozenset(tok_cols)
    CSEG_OFF, M2_OFF, CTOK_OFF, M1_OFF, PE_OFF, C, _ = _layout(tok_cols, dev_bcast)
    NV = 2 if tok_cols else 1
    cvec = None
    if dev_bcast:
        cvec = np.empty((1, NV * D), dtype=_DEV_NP)
        cvec[0, :D] = c_seg
        if tok_cols:
            cvec[0, D:] = c_tok
    in_maps = []
    for pe_sl, m1, m2 in per_core:
        consts = np.empty((P, C), dtype=_DEV_NP)
        consts[:, M2_OFF:M2_OFF + J] = m2
        if tok_cols:
            consts[:, M1_OFF:M1_OFF + J] = m1
        if not dev_bcast:
            consts[:, CSEG_OFF:CSEG_OFF + D] = c_seg
            if tok_cols:
                consts[:, CTOK_OFF:CTOK_OFF + D] = c_tok
        consts[:, PE_OFF:PE_OFF + T_TILES * D] = pe_sl
        m = {"consts": consts}
        if dev_bcast:
            m["cvec"] = cvec
        in_maps.append(m)
    return in_maps, tok_cols


# -------------------------------------------------- tile variant (fallback) —

TPE_OFF = 0
TCSEG_OFF = T_TILES * D
TM2_OFF = TCSEG_OFF + D
TC_COMMON = TM2_OFF + J
TCTOK_OFF = TC_COMMON
TM1_OFF = TCTOK_OFF + D
TC_FULL = TM1_OFF + J


def _build_tile(tok_cols: frozenset) -> "bass.Bass":
    C = TC_FULL if tok_cols else TC_COMMON
    # Bacc (not plain Bass): its compile() splits multi-semaphore waits into
    # event semaphores — TRN2 allows at most one inline wait per instruction.
    nc = bacc.Bacc("TRN2", target_bir_lowering=False)
    const_d = nc.dram_tensor("consts", [P, C], _DEV_DT, kind="ExternalInput")
    out_d = nc.dram_tensor("out", [B * S_SH, D], _F32, kind="ExternalOutput")
    mult, add = mybir.AluOpType.mult, mybir.AluOpType.add

    with tile.TileContext(nc) as tc:
        with (
            tc.tile_pool(name="const", bufs=1) as cpool,
            tc.tile_pool(name="outp", bufs=8) as opool,
        ):
            c_t = cpool.tile([P, C], _F32)
            nc.sync.dma_start(c_t[:], const_d[:])
            cseg_t = c_t[:, TCSEG_OFF:TCSEG_OFF + D]
            ctok_t = c_t[:, TCTOK_OFF:TCTOK_OFF + D] if tok_cols else None

            for b in range(B):
                for t in range(T_TILES):
                    j = b * T_TILES + t
                    o = opool.tile([P, D], _F32, tag="out")
                    pe_slice = c_t[:, TPE_OFF + t * D:TPE_OFF + (t + 1) * D]
                    m2_col = c_t[:, TM2_OFF + j:TM2_OFF + j + 1]
                    if j in tok_cols:
                        m1_col = c_t[:, TM1_OFF + j:TM1_OFF + j + 1]
                        nc.vector.scalar_tensor_tensor(
                            o[:], ctok_t, m1_col, pe_slice, op0=mult, op1=add,
                        )
                        nc.vector.scalar_tensor_tensor(
                            o[:], cseg_t, m2_col, o[:], op0=mult, op1=add,
                        )
                    else:
                        nc.vector.scalar_tensor_tensor(
                            o[:], cseg_t, m2_col, pe_slice, op0=mult, op1=add,
                        )
                    nc.sync.dma_start(out_d[j * P:(j + 1) * P, :], o[:])
    nc.finalize()
    return nc


def _prepare_tile(inputs: dict):
    ids = np.asarray(inputs["input_ids"])
    seg = np.asarray(inputs["segment_label"])
    W_tok = np.asarray(inputs["W_tok"], dtype=np.float32)
    b_tok = np.asarray(inputs["b_tok"], dtype=np.float32)
    W_seg = np.asarray(inputs["W_seg"], dtype=np.float32)
    b_seg = np.asarray(inputs["b_seg"], dtype=np.float32)
    pe = np.asarray(inputs["pe"], dtype=np.float32).reshape(SEQ, D)

    c_tok = (W_tok[:, 0] + b_tok).astype(np.float32)
    c_seg = (W_seg[:, 0] + b_seg).astype(np.float32)
    m1_full = (ids == 0).astype(np.float32)
    m2_full = (seg == 0).astype(np.float32)

    per_core = []
    tok_cols = set()
    for c in range(N_CORES):
        sl = slice(c * S_SH, (c + 1) * S_SH)
        m1 = m1_full[:, sl].reshape(B, T_TILES, P).transpose(2, 0, 1).reshape(P, J)
        m2 = m2_full[:, sl].reshape(B, T_TILES, P).transpose(2, 0, 1).reshape(P, J)
        pe_sl = pe[sl].reshape(T_TILES, P, D).transpose(1, 0, 2).reshape(P, T_TILES * D)
        tok_cols.update(np.nonzero(m1.any(axis=0))[0].tolist())
        per_core.append((pe_sl, m1, m2))

    need_tok = bool(tok_cols)
    C = TC_FULL if need_tok else TC_COMMON
    in_maps = []
    for pe_sl, m1, m2 in per_core:
        consts = np.empty((P, C), dtype=np.float32)
        consts[:, TPE_OFF:TPE_OFF + T_TILES * D] = pe_sl
        consts[:, TCSEG_OFF:TCSEG_OFF + D] = c_seg
        consts[:, TM2_OFF:TM2_OFF + J] = m2
        if need_tok:
            consts[:, TCTOK_OFF:TCTOK_OFF + D] = c_tok
            consts[:, TM1_OFF:TM1_OFF + J] = m1
        in_maps.append({"consts": consts})
    return in_maps, frozenset(tok_cols)


# ------------------------------------------------------------------- entry —

def kernel(**inputs) -> np.ndarray:
    global LAST_RESULTS
    impl = os.environ.get("BASS_KERNEL_IMPL", "raw")
    if impl == "raw":
        # NOTE: pool_split compiles in CoreSim but walrus rejects
        # TensorScalarPtr on the Pool engine (NCC_IXCG966) — keep off.
        pool_split = bool(int(os.environ.get("BASS_KERNEL_POOL_SPLIT", "0")))
        dev_bcast = bool(int(os.environ.get("BASS_KERNEL_DEV_BCAST", "0")))
        in_maps, tok_cols = _prepare_raw(inputs, dev_bcast=dev_bcast)
        key = ("raw", pool_split, dev_bcast, tok_cols)
        def builder(tc):
            return _build_raw(tc, pool_split=pool_split, dev_bcast=dev_bcast)
    else:
        in_maps, tok_cols = _prepare_tile(inputs)
        key = ("tile", tok_cols)
        builder = _build_tile
    # SPMD: one program for all cores; the tok op is emitted for any column
    # that needs it on any core (a zero mask column makes it the identity).
    if key not in _prog_cache:
        _prog_cache[key] = builder(tok_cols)
    nc = _prog_cache[key]

    trace = bool(int(os.environ.get("BASS_KERNEL_TRACE", "0")))
    try:
        res = run_bass_kernel_spmd(
            nc, in_maps, list(range(N_CORES)), trace=trace,
            trace_cores=list(range(N_CORES)) if trace else None,
        )
    except ModuleNotFoundError:
        # axon builds without the NTFF profile hook (antenv.axon_hooks)
        # crash when tracing is requested (e.g. BASS_TRACE=1 in the env);
        # degrade to an untraced run rather than failing the kernel.
        os.environ["BASS_NEVER_TRACE"] = "1"
        res = run_bass_kernel_spmd(nc, in_maps, list(range(N_CORES)), trace=False)
    LAST_RESULTS = res

    out = np.empty((B, SEQ, D), dtype=np.float32)
    for c in range(N_CORES):
        out[:, c * S_SH:(c + 1) * S_SH, :] = (
            np.asarray(res.results[c]["out"]).astype(np.float32).reshape(B, S_SH, D)
        )
    return out



# revision 4
# speedup vs baseline: 1.2424x; 1.2169x over previous
"""Trainium2 Bass kernel for nn_BeBertEmbedding (self-contained).

Math: the reference's semantic_embed(ids, W, b, pad=0) is
    where(ids==0, take(W.T, ids) + b, zeros)
so the only table row that survives is W[:, 0], and the whole module is
    out[b,s,:] = pe[s,:] + (ids[b,s]==0)*(W_tok[:,0]+b_tok)
                         + (seg[b,s]==0)*(W_seg[:,0]+b_seg)

Sharding: sequence-parallel across 8 cores (256 positions/core, all 16
batches; each core writes a disjoint [16, 256, 768] slice, no collectives).

Everything on device is fp16 (tolerance is rel 2e-2; fp16 keeps us ~7e-4):
halves HBM traffic vs f32 and unlocks the DVE 16-bit perf modes.  The host
upcasts the returned fp16 shard to float32.

Per core the program is raw Bass, 32 output tiles [128 tokens, 768] resident
in SBUF at once (49KB/partition, no slot recycling):
  * corr stage: one DVE tensor_scalar per tile (cseg * m2[p]) -- fp16 data
    with an fp32 per-partition mask scalar runs in the 4x DVE mode (260ns
    vs 860ns for the old fused scalar_tensor_tensor, which has no fast
    mode); the rare tiles with a zero token id get one extra STT.
  * combine stage: per chunk of 1-5 tiles one fat tensor_tensor adds the
    seq-tile's pe broadcast across the chunk (stride-0 mid-dim AP) -- 2x
    DVE mode, and the fat shape amortizes the per-op init.
  * out-DMAs: one multi-tile DMA per chunk (regular 3-dim AP over
    out[b*256 + t*128 + p, d]), alternating the SP and ACT HW-DGE rings.
  * input loads split across both rings (masks+pe on SP, c-vectors on ACT)
    so the first corr op starts as early as the per-DMA latency allows.
  * chunk sizes taper (5,5,3,2,1): fat early chunks for TT-init economy,
    tiny final chunk to shorten the last TT -> last DMA -> drain tail.

Cost-model time 26.7us/core (was 33.2us for the f32 STT baseline): head
~2.4us (DMA fixed latency), DVE busy ~21.7us (the 2-pass 4x/2x floor is
~21.2), tail ~2.6us.  DVE is the bottleneck; both DMA rings sit at ~9.5us
busy each, well under it.  Rejected: ACT corr offload (activation init
makes every op 2.2us), PE rank-1 corr into PSUM (any PSUM operand drops
DVE to 1x, 925ns/tile), copy_predicated select (no fast mode, 860ns),
HBM-side scatter/accum tricks (double-write real HBM traffic).
"""

import contextlib
import os
import sys

import numpy as np

try:
    from concourse import bass, mybir
    from concourse.bass_utils import run_bass_kernel_spmd
except ImportError:
    for _p in ("/opt/trn_rl_repo", "/root/.axon_site/_ro/trn_rl_repo"):
        if os.path.isdir(_p) and _p not in sys.path:
            sys.path.insert(0, _p)
            break
    from concourse import bass, mybir
    from concourse.bass_utils import run_bass_kernel_spmd

N_CORES = 8
B, SEQ, D = 16, 2048, 768
S_SH = SEQ // N_CORES        # 256 sequence positions per core
P = 128                      # partitions
T_TILES = S_SH // P          # 2 seq tiles per core
J = B * T_TILES              # 32 output tiles per core

_F16 = mybir.dt.float16
_F32 = mybir.dt.float32

# chunk sizes (in tiles) per seq-tile t; must sum to 16.  Tapered: see
# module docstring.
CHUNKS = (5, 5, 3, 2, 1)

_prog_cache: dict = {}
LAST_RESULTS = None          # BassKernelResults of the most recent run


def _chunk_list():
    out = []
    for t in range(T_TILES):
        b0 = 0
        for nb in CHUNKS:
            out.append((t, b0, nb))
            b0 += nb
        assert b0 == B
    return out


def _build(tok_cols: frozenset) -> "bass.Bass":
    need_tok = bool(tok_cols)
    # consts fp16: [cseg 768 | (ctok 768) | pe0 768 | pe1 768]
    CSEG = 0
    CTOK = D if need_tok else None
    PE = (2 * D) if need_tok else D
    C = PE + T_TILES * D
    NM = 2 * J if need_tok else J      # fp32 mask cols: m2 | (m1)

    nc = bass.Bass("TRN2")
    const_d = nc.dram_tensor("consts", [P, C], _F16, kind="ExternalInput")
    mask_d = nc.dram_tensor("masks", [P, NM], _F32, kind="ExternalInput")
    out_d = nc.dram_tensor("out", [B * S_SH, D], _F16, kind="ExternalOutput")
    mult, add = mybir.AluOpType.mult, mybir.AluOpType.add

    chunks = _chunk_list()

    with contextlib.ExitStack() as stack:
        c_t = stack.enter_context(nc.sbuf_tensor([P, C], _F16))
        m_t = stack.enter_context(nc.sbuf_tensor([P, NM], _F32))
        # slot s = t*16 + b; slots of one t contiguous so a chunk is one AP
        slots = stack.enter_context(nc.sbuf_tensor([P, J * D], _F16))
        m_sem = stack.enter_context(nc.semaphore("m_sem"))
        c_sem = stack.enter_context(nc.semaphore("c_sem"))
        b_sem = stack.enter_context(nc.semaphore("b_sem"))
        v_sem = stack.enter_context(nc.semaphore("v_sem"))
        o_sem = stack.enter_context(nc.semaphore("o_sem"))
        # DVE pipeline is deep: ops reading a same-engine predecessor's
        # output need an explicit retire guard (ts_sem counts corr ops).
        ts_sem = stack.enter_context(nc.semaphore("ts_sem"))
        block = stack.enter_context(nc.Block())

        cseg = c_t[:, CSEG:CSEG + D]
        ctok = c_t[:, CTOK:CTOK + D] if need_tok else None

        def issue_out(eng, parity):
            # out-DMA per chunk k waits v_sem >= k+1 (fat TT k retired);
            # dst element (p, b, d) -> row b*256 + t*128 + p, col d
            out_v = out_d[:, :].rearrange(
                "(b t p) d -> p t b d", b=B, t=T_TILES)
            for k, (t, b0, nb) in enumerate(chunks):
                if k % 2 != parity:
                    continue
                s0 = t * B + b0
                src = slots[:, s0 * D:(s0 + nb) * D].rearrange(
                    "p (s d) -> p s d", s=nb)
                eng.dma_start(out_v[:, t, b0:b0 + nb, :], src) \
                    ._wait_ge(v_sem, k + 1).then_inc(o_sem, 16)

        @block.sync
        def _(sync):
            # SP ring: masks (tiny, gates the first corr op) then pe
            sync.dma_start(m_t[:], mask_d[:]).then_inc(m_sem, 16)
            sync.dma_start(c_t[:, PE:], const_d[:, PE:]).then_inc(b_sem, 16)
            issue_out(sync, 0)

        @block.scalar
        def _(scalar):
            # ACT ring: c-vectors, in parallel with the SP loads
            scalar.dma_start(c_t[:, :PE], const_d[:, :PE]).then_inc(c_sem, 16)
            issue_out(scalar, 1)

        @block.vector
        def _(vector):
            nc.vector.wait_ge(m_sem, 16)
            nc.vector.wait_ge(c_sem, 16)
            waited_b = False
            ncorr = 0
            for k, (t, b0, nb) in enumerate(chunks):
                for b in range(b0, b0 + nb):
                    j = b * T_TILES + t      # mask column index
                    s = t * B + b
                    sl = slots[:, s * D:(s + 1) * D]
                    nc.vector.tensor_scalar(
                        sl, cseg, scalar1=m_t[:, j:j + 1], op0=mult,
                        scalar2=None).then_inc(ts_sem, 1)
                    ncorr += 1
                    if j in tok_cols:
                        nc.vector.scalar_tensor_tensor(
                            sl, ctok, m_t[:, J + j:J + j + 1], sl,
                            op0=mult, op1=add,
                        )._wait_ge(ts_sem, ncorr).then_inc(ts_sem, 1)
                        ncorr += 1
                if not waited_b:
                    nc.vector.wait_ge(b_sem, 16)
                    waited_b = True
                s0 = t * B + b0
                grp = slots[:, s0 * D:(s0 + nb) * D].rearrange(
                    "p (s d) -> p s d", s=nb)
                pe_b = c_t[:, PE + t * D:PE + (t + 1) * D].unsqueeze(1) \
                    .broadcast_to([P, nb, D])
                nc.vector.tensor_tensor(grp, grp, pe_b, op=add) \
                    ._wait_ge(ts_sem, ncorr).then_inc(v_sem, 1)

    nc.finalize()
    return nc


def _prepare(inputs: dict):
    ids = np.asarray(inputs["input_ids"])
    seg = np.asarray(inputs["segment_label"])
    W_tok = np.asarray(inputs["W_tok"], dtype=np.float32)
    b_tok = np.asarray(inputs["b_tok"], dtype=np.float32)
    W_seg = np.asarray(inputs["W_seg"], dtype=np.float32)
    b_seg = np.asarray(inputs["b_seg"], dtype=np.float32)
    pe = np.asarray(inputs["pe"], dtype=np.float32).reshape(SEQ, D)

    c_tok = (W_tok[:, 0] + b_tok).astype(np.float16)
    c_seg = (W_seg[:, 0] + b_seg).astype(np.float16)
    m1_full = (ids == 0).astype(np.float32)
    m2_full = (seg == 0).astype(np.float32)

    per_core = []
    tok_cols = set()
    for c in range(N_CORES):
        sl = slice(c * S_SH, (c + 1) * S_SH)
        # [B, S_SH] -> [P, J] with column j = b*T_TILES + t, partition p
        m1 = m1_full[:, sl].reshape(B, T_TILES, P).transpose(2, 0, 1).reshape(P, J)
        m2 = m2_full[:, sl].reshape(B, T_TILES, P).transpose(2, 0, 1).reshape(P, J)
        pe_sl = pe[sl].reshape(T_TILES, P, D).transpose(1, 0, 2) \
            .reshape(P, T_TILES * D).astype(np.float16)
        tok_cols.update(np.nonzero(m1.any(axis=0))[0].tolist())
        per_core.append((pe_sl, m1, m2))

    # SPMD: one program for all cores; a tile gets the tok STT if any core
    # needs it (a zero mask column makes it the identity elsewhere).
    tok_cols = frozenset(tok_cols)
    need_tok = bool(tok_cols)
    CSEG = 0
    CTOK = D if need_tok else None
    PE = (2 * D) if need_tok else D
    C = PE + T_TILES * D
    NM = 2 * J if need_tok else J

    in_maps = []
    for pe_sl, m1, m2 in per_core:
        consts = np.empty((P, C), dtype=np.float16)
        consts[:, CSEG:CSEG + D] = c_seg
        if need_tok:
            consts[:, CTOK:CTOK + D] = c_tok
        consts[:, PE:] = pe_sl
        masks = np.empty((P, NM), dtype=np.float32)
        masks[:, :J] = m2
        if need_tok:
            masks[:, J:] = m1
        in_maps.append({"consts": consts, "masks": masks})
    return in_maps, tok_cols


def kernel(**inputs) -> np.ndarray:
    global LAST_RESULTS
    in_maps, tok_cols = _prepare(inputs)
    key = ("v2", CHUNKS, tok_cols)
    if key not in _prog_cache:
        _prog_cache[key] = _build(tok_cols)
    nc = _prog_cache[key]

    trace = bool(int(os.environ.get("BASS_KERNEL_TRACE", "0")))
    try:
        res = run_bass_kernel_spmd(
            nc, in_maps, list(range(N_CORES)), trace=trace,
            trace_cores=list(range(N_CORES)) if trace else None,
        )
    except ModuleNotFoundError:
        # axon builds without the NTFF profile hook (antenv.axon_hooks)
        # crash when tracing is requested (e.g. BASS_TRACE=1 in the env);
        # degrade to an untraced run rather than failing the kernel.
        os.environ["BASS_NEVER_TRACE"] = "1"
        res = run_bass_kernel_spmd(nc, in_maps, list(range(N_CORES)), trace=False)
    LAST_RESULTS = res

    out = np.empty((B, SEQ, D), dtype=np.float32)
    for c in range(N_CORES):
        out[:, c * S_SH:(c + 1) * S_SH, :] = (
            np.asarray(res.results[c]["out"]).astype(np.float32)
            .reshape(B, S_SH, D)
        )
    return out


# revision 13
# speedup vs baseline: 1.7629x; 1.4189x over previous
"""Trainium2 Bass kernel for nn_BeBertEmbedding (self-contained).

Math: the reference's semantic_embed(ids, W, b, pad=0) is
    where(ids==0, take(W.T, ids) + b, zeros)
so the only table row that survives is W[:, 0], and the whole module is
    out[b,s,:] = pe[s,:] + (ids[b,s]==0)*(W_tok[:,0]+b_tok)
                         + (seg[b,s]==0)*(W_seg[:,0]+b_seg)

Sharding: sequence-parallel across 8 cores (256 positions/core, all 16
batches; each core writes a disjoint [16, 256, 768] slice, no collectives).

Everything on device is fp16 (tolerance is rel 2e-2; fp16 keeps us ~7e-4):
halves HBM traffic vs f32 and unlocks the DVE 16-bit perf modes.  The host
upcasts the returned fp16 shard to float32.

Per core the program is raw Bass, 32 output tiles [128 tokens, 768] resident
in SBUF at once (49KB/partition, no slot recycling):
  * corr stage: one DVE tensor_scalar per tile (cseg * m2[p]) -- fp16 data
    with an fp32 per-partition mask scalar runs in the 4x DVE mode (260ns
    vs 860ns for the old fused scalar_tensor_tensor, which has no fast
    mode); the rare tiles with a zero token id get one extra STT.
  * combine stage: per chunk of 1-5 tiles one fat tensor_tensor adds the
    seq-tile's pe broadcast across the chunk (stride-0 mid-dim AP) -- 2x
    DVE mode, and the fat shape amortizes the per-op init.
  * out-DMAs: one multi-tile DMA per chunk (regular 3-dim AP over
    out[b*256 + t*128 + p, d]), alternating the SP and ACT HW-DGE rings.
  * input loads split across both rings (masks+pe on SP, c-vectors on ACT)
    so the first corr op starts as early as the per-DMA latency allows.
  * chunk sizes taper (5,5,3,2,1): fat early chunks for TT-init economy,
    tiny final chunk to shorten the last TT -> last DMA -> drain tail.

Cost-model time 26.7us/core (was 33.2us for the f32 STT baseline): head
~2.4us (DMA fixed latency), DVE busy ~21.7us (the 2-pass 4x/2x floor is
~21.2), tail ~2.6us.  DVE is the bottleneck; both DMA rings sit at ~9.5us
busy each, well under it.  Rejected: ACT corr offload (activation init
makes every op 2.2us), PE rank-1 corr into PSUM (any PSUM operand drops
DVE to 1x, 925ns/tile), copy_predicated select (no fast mode, 860ns),
HBM-side scatter/accum tricks (double-write real HBM traffic).
"""

import contextlib
import os
import sys

import numpy as np

try:
    from concourse import bass, mybir
    from concourse.bass_utils import run_bass_kernel_spmd
except ImportError:
    for _p in ("/opt/trn_rl_repo", "/root/.axon_site/_ro/trn_rl_repo"):
        if os.path.isdir(_p) and _p not in sys.path:
            sys.path.insert(0, _p)
            break
    from concourse import bass, mybir
    from concourse.bass_utils import run_bass_kernel_spmd

N_CORES = 8
B, SEQ, D = 16, 2048, 768
S_SH = SEQ // N_CORES        # 256 sequence positions per core
P = 128                      # partitions
T_TILES = S_SH // P          # 2 seq tiles per core
J = B * T_TILES              # 32 output tiles per core

_F16 = mybir.dt.float16
_F32 = mybir.dt.float32

# Combine-stage work split: the fat TT chunks per seq-tile t, with an
# owner engine.  Pool (GPSIMD) adds at 640ns/tile in parallel with DVE
# (400ns/tile + init), so 20 tiles go to Pool and 12 stay on DVE; both
# streams then run ~13us.  All corr TS ops stay on DVE (TensorScalarPtr
# is walrus-rejected on Pool).  The per-t segment pattern interleaves
# pool-TS groups with dve TS+TT work, sized so the Pool stream (which
# consumes 640ns/tile against DVE's 260ns/tile TS production) never
# starves while dve-chunk completions spread across the run -- the
# chunk order below is also the out-DMA issue order per ring, keeping
# the in-order rings free of head-of-line blocking.
# Global segment order (owner, t, nb): cross-t interleaving keeps the Pool
# stream fed across the t boundary; both streams taper to 1-tile final
# chunks so the last TT -> last DMA tail is short.
SEGMENTS = (
    ("pool", 0, 2), ("pool", 0, 4), ("dve", 0, 2), ("pool", 0, 4),
    ("dve", 0, 2), ("pool", 1, 4), ("dve", 0, 2), ("pool", 1, 2),
    ("dve", 1, 2), ("pool", 1, 2), ("dve", 1, 2), ("pool", 1, 1),
    ("dve", 1, 1), ("pool", 1, 1), ("dve", 1, 1),
)

_prog_cache: dict = {}
LAST_RESULTS = None          # BassKernelResults of the most recent run


def _chunk_list():
    """[(owner, t, b0, nb), ...] in TS-emission order; per t, pool owns
    the low b range, dve the high b."""
    out = []
    bp = [0] * T_TILES
    npool = [sum(nb for o, t_, nb in SEGMENTS if o == "pool" and t_ == t)
             for t in range(T_TILES)]
    bd = list(npool)
    for own, t, nb in SEGMENTS:
        if own == "pool":
            out.append((own, t, bp[t], nb))
            bp[t] += nb
        else:
            out.append((own, t, bd[t], nb))
            bd[t] += nb
    for t in range(T_TILES):
        assert bd[t] == B and bp[t] == npool[t]
    return out


def _ring_plan(chunks, tok_cols):
    """Static completion-time estimate per chunk (cost-model arithmetic),
    then greedy earliest-finish assignment of the out-DMAs to the two
    HW-DGE rings.  Returns [(ring, chunk_idx), ...] per ring in issue
    order."""
    TS_N, TT_DVE, TT_DVE_INIT, TT_POOL, STT_N = 260.0, 400.0, 60.0, 640.0, 860.0
    t_head = 2417.0                  # masks-DMA completion gates first TS
    pe_done = (3009.0, 3601.0)       # pe0/pe1 completion (SP ring queue)
    dve = t_head
    pool = 0.0
    ts_done = {}
    ready = {}
    for k, (own, t, b0, nb) in enumerate(chunks):
        for b in range(b0, b0 + nb):
            dve += TS_N
            if b * T_TILES + t in tok_cols:
                dve += STT_N
        ts_done[k] = dve
        if own == "dve":
            dve = max(dve, pe_done[t]) + nb * TT_DVE + TT_DVE_INIT
            ready[k] = dve
    for k, (own, t, b0, nb) in enumerate(chunks):
        if own != "pool":
            continue
        pool = max(pool, ts_done[k], pe_done[t]) + nb * TT_POOL
        ready[k] = pool
    order = sorted(range(len(chunks)), key=lambda k: ready[k])
    rings = {0: [], 1: []}
    free = {0: 0.0, 1: 0.0}
    for k in order:
        nb = chunks[k][3]
        busy = max(500.0, nb * 592.0)
        r = min((0, 1), key=lambda r: max(free[r], ready[k]) + busy)
        free[r] = max(free[r], ready[k]) + busy
        rings[r].append(k)
    return rings


def _build(tok_cols: frozenset) -> "bass.Bass":
    need_tok = bool(tok_cols)
    # consts fp16: [cseg 768 | (ctok 768) | pe0 768 | pe1 768]
    CSEG = 0
    CTOK = D if need_tok else None
    PE = (2 * D) if need_tok else D
    C = PE + T_TILES * D
    NM = 2 * J if need_tok else J      # fp32 mask cols: m2 | (m1)

    nc = bass.Bass("TRN2")
    const_d = nc.dram_tensor("consts", [P, C], _F16, kind="ExternalInput")
    mask_d = nc.dram_tensor("masks", [P, NM], _F32, kind="ExternalInput")
    out_d = nc.dram_tensor("out", [B * S_SH, D], _F16, kind="ExternalOutput")
    mult, add = mybir.AluOpType.mult, mybir.AluOpType.add

    chunks = _chunk_list()

    with contextlib.ExitStack() as stack:
        c_t = stack.enter_context(nc.sbuf_tensor([P, C], _F16))
        m_t = stack.enter_context(nc.sbuf_tensor([P, NM], _F32))
        # slot s = t*16 + b; slots of one t contiguous so a chunk is one AP
        slots = stack.enter_context(nc.sbuf_tensor([P, J * D], _F16))
        m_sem = stack.enter_context(nc.semaphore("m_sem"))
        c_sem = stack.enter_context(nc.semaphore("c_sem"))
        b_sems = [stack.enter_context(nc.semaphore(f"b{t}_sem"))
                  for t in range(T_TILES)]
        v_sem = stack.enter_context(nc.semaphore("v_sem"))
        p_sem = stack.enter_context(nc.semaphore("p_sem"))
        o_sem = stack.enter_context(nc.semaphore("o_sem"))
        # DVE pipeline is deep: ops reading a same-engine predecessor's
        # output need an explicit retire guard; ts_sem counts corr-op
        # retires and also carries the cross-engine TS->PoolTT ordering.
        ts_sem = stack.enter_context(nc.semaphore("ts_sem"))
        block = stack.enter_context(nc.Block())

        cseg = c_t[:, CSEG:CSEG + D]
        ctok = c_t[:, CTOK:CTOK + D] if need_tok else None

        # corr-op retire count after each chunk's TS group (filled while
        # emitting the vector block, read by pool/DMA emitters)
        ts_after: dict = {}
        own_idx: dict = {}
        nv = np_ = 0
        for k, (own, t, b0, nb) in enumerate(chunks):
            if own == "dve":
                nv += 1
                own_idx[k] = nv
            else:
                np_ += 1
                own_idx[k] = np_

        def grp_ap(t, b0, nb):
            s0 = t * B + b0
            return slots[:, s0 * D:(s0 + nb) * D].rearrange(
                "p (s d) -> p s d", s=nb)

        def pe_bcast(t, nb):
            return c_t[:, PE + t * D:PE + (t + 1) * D].unsqueeze(1) \
                .broadcast_to([P, nb, D])

        ring_plan = _ring_plan(chunks, tok_cols)

        def issue_out(eng, ring):
            # out-DMA per chunk waits its owner's TT-retire count;
            # dst element (p, b, d) -> row b*256 + t*128 + p, col d
            out_v = out_d[:, :].rearrange(
                "(b t p) d -> p t b d", b=B, t=T_TILES)
            for k in ring_plan[ring]:
                own, t, b0, nb = chunks[k]
                done = (v_sem if own == "dve" else p_sem, own_idx[k])
                eng.dma_start(out_v[:, t, b0:b0 + nb, :],
                              grp_ap(t, b0, nb)) \
                    ._wait_ge(*done).then_inc(o_sem, 16)

        @block.sync
        def _(sync):
            # SP ring: masks (tiny, gates the first corr op), then the two
            # pe halves -- pe0 lands early so the TT streams start sooner
            sync.dma_start(m_t[:], mask_d[:]).then_inc(m_sem, 16)
            for t in range(T_TILES):
                sync.dma_start(c_t[:, PE + t * D:PE + (t + 1) * D],
                               const_d[:, PE + t * D:PE + (t + 1) * D]) \
                    .then_inc(b_sems[t], 16)
            issue_out(sync, 0)

        @block.scalar
        def _(scalar):
            # ACT ring: c-vectors, in parallel with the SP loads
            scalar.dma_start(c_t[:, :PE], const_d[:, :PE]).then_inc(c_sem, 16)
            issue_out(scalar, 1)

        @block.vector
        def _(vector):
            nc.vector.wait_ge(m_sem, 16)
            nc.vector.wait_ge(c_sem, 16)
            waited_b = [False] * T_TILES
            ncorr = 0
            for k, (own, t, b0, nb) in enumerate(chunks):
                for b in range(b0, b0 + nb):
                    j = b * T_TILES + t      # mask column index
                    s = t * B + b
                    sl = slots[:, s * D:(s + 1) * D]
                    nc.vector.tensor_scalar(
                        sl, cseg, scalar1=m_t[:, j:j + 1], op0=mult,
                        scalar2=None).then_inc(ts_sem, 1)
                    ncorr += 1
                    if j in tok_cols:
                        nc.vector.scalar_tensor_tensor(
                            sl, ctok, m_t[:, J + j:J + j + 1], sl,
                            op0=mult, op1=add,
                        )._wait_ge(ts_sem, ncorr).then_inc(ts_sem, 1)
                        ncorr += 1
                ts_after[k] = ncorr
                if own != "dve":
                    continue
                if not waited_b[t]:
                    nc.vector.wait_ge(b_sems[t], 16)
                    waited_b[t] = True
                nc.vector.tensor_tensor(
                    grp_ap(t, b0, nb), grp_ap(t, b0, nb), pe_bcast(t, nb),
                    op=add)._wait_ge(ts_sem, ncorr).then_inc(v_sem, 1)

        @block.gpsimd
        def _(gpsimd):
            waited_b = [False] * T_TILES
            for k, (own, t, b0, nb) in enumerate(chunks):
                if own != "pool":
                    continue
                if not waited_b[t]:
                    gpsimd.wait_ge(b_sems[t], 16)
                    waited_b[t] = True
                nc.gpsimd.tensor_tensor(
                    grp_ap(t, b0, nb), grp_ap(t, b0, nb), pe_bcast(t, nb),
                    op=add)._wait_ge(ts_sem, ts_after[k]).then_inc(p_sem, 1)

    nc.finalize()
    return nc


def _prepare(inputs: dict):
    ids = np.asarray(inputs["input_ids"])
    seg = np.asarray(inputs["segment_label"])
    W_tok = np.asarray(inputs["W_tok"], dtype=np.float32)
    b_tok = np.asarray(inputs["b_tok"], dtype=np.float32)
    W_seg = np.asarray(inputs["W_seg"], dtype=np.float32)
    b_seg = np.asarray(inputs["b_seg"], dtype=np.float32)
    pe = np.asarray(inputs["pe"], dtype=np.float32).reshape(SEQ, D)

    c_tok = (W_tok[:, 0] + b_tok).astype(np.float16)
    c_seg = (W_seg[:, 0] + b_seg).astype(np.float16)
    m1_full = (ids == 0).astype(np.float32)
    m2_full = (seg == 0).astype(np.float32)

    per_core = []
    tok_cols = set()
    for c in range(N_CORES):
        sl = slice(c * S_SH, (c + 1) * S_SH)
        # [B, S_SH] -> [P, J] with column j = b*T_TILES + t, partition p
        m1 = m1_full[:, sl].reshape(B, T_TILES, P).transpose(2, 0, 1).reshape(P, J)
        m2 = m2_full[:, sl].reshape(B, T_TILES, P).transpose(2, 0, 1).reshape(P, J)
        pe_sl = pe[sl].reshape(T_TILES, P, D).transpose(1, 0, 2) \
            .reshape(P, T_TILES * D).astype(np.float16)
        tok_cols.update(np.nonzero(m1.any(axis=0))[0].tolist())
        per_core.append((pe_sl, m1, m2))

    # SPMD: one program for all cores; a tile gets the tok STT if any core
    # needs it (a zero mask column makes it the identity elsewhere).
    tok_cols = frozenset(tok_cols)
    need_tok = bool(tok_cols)
    CSEG = 0
    CTOK = D if need_tok else None
    PE = (2 * D) if need_tok else D
    C = PE + T_TILES * D
    NM = 2 * J if need_tok else J

    in_maps = []
    for pe_sl, m1, m2 in per_core:
        consts = np.empty((P, C), dtype=np.float16)
        consts[:, CSEG:CSEG + D] = c_seg
        if need_tok:
            consts[:, CTOK:CTOK + D] = c_tok
        consts[:, PE:] = pe_sl
        masks = np.empty((P, NM), dtype=np.float32)
        masks[:, :J] = m2
        if need_tok:
            masks[:, J:] = m1
        in_maps.append({"consts": consts, "masks": masks})
    return in_maps, tok_cols


def kernel(**inputs) -> np.ndarray:
    global LAST_RESULTS
    in_maps, tok_cols = _prepare(inputs)
    key = ("v3", SEGMENTS, tok_cols)
    if key not in _prog_cache:
        _prog_cache[key] = _build(tok_cols)
    nc = _prog_cache[key]

    trace = bool(int(os.environ.get("BASS_KERNEL_TRACE", "0")))
    try:
        res = run_bass_kernel_spmd(
            nc, in_maps, list(range(N_CORES)), trace=trace,
            trace_cores=list(range(N_CORES)) if trace else None,
        )
    except ModuleNotFoundError:
        # axon builds without the NTFF profile hook (antenv.axon_hooks)
        # crash when tracing is requested (e.g. BASS_TRACE=1 in the env);
        # degrade to an untraced run rather than failing the kernel.
        os.environ["BASS_NEVER_TRACE"] = "1"
        res = run_bass_kernel_spmd(nc, in_maps, list(range(N_CORES)), trace=False)
    LAST_RESULTS = res

    out = np.empty((B, SEQ, D), dtype=np.float32)
    for c in range(N_CORES):
        out[:, c * S_SH:(c + 1) * S_SH, :] = (
            np.asarray(res.results[c]["out"]).astype(np.float32)
            .reshape(B, S_SH, D)
        )
    return out


# revision 24
# speedup vs baseline: 1.7813x; 1.0105x over previous
"""Trainium2 Bass kernel for nn_BeBertEmbedding (self-contained).

Math: the reference's semantic_embed(ids, W, b, pad=0) is
    where(ids==0, take(W.T, ids) + b, zeros)
so the only table row that survives is W[:, 0], and the whole module is
    out[b,s,:] = pe[s,:] + (ids[b,s]==0)*(W_tok[:,0]+b_tok)
                         + (seg[b,s]==0)*(W_seg[:,0]+b_seg)

Sharding: sequence-parallel across 8 cores (256 positions/core, all 16
batches; each core writes a disjoint [16, 256, 768] slice, no collectives).

Everything on device is fp16 (tolerance is rel 2e-2; fp16 keeps us ~7e-4):
halves HBM traffic vs f32 and unlocks the DVE 16-bit perf modes.  The host
upcasts the returned fp16 shard to float32.

Per core the program is raw Bass, 32 output tiles [128 tokens, 768] resident
in SBUF at once (49KB/partition, no slot recycling):
  * corr stage (DVE): one tensor_scalar per tile (cseg * m2[p]) -- fp16
    data with an fp32 per-partition mask scalar runs in the 4x DVE mode
    (260ns vs 860ns for the fused scalar_tensor_tensor, which has no fast
    mode); the rare tiles with a zero token id get one extra STT.
  * combine stage (DVE + Pool in parallel): per chunk of 1-4 tiles one fat
    tensor_tensor adds the seq-tile's pe broadcast across the chunk
    (stride-0 mid-dim AP).  20 tiles go to the otherwise-idle GPSIMD at
    640ns/tile (the documented 153 Gelem/s rate; plain TensorTensor only
    -- TensorScalarPtr on Pool is walrus-rejected, NCC_IXCG966) and 12
    stay on DVE at 2x mode (~400ns/tile); with DVE also carrying all 32
    corr ops, both streams run ~13us and finish together.  SEGMENTS
    interleaves pool-TS groups with dve TS+TT work, cross-t, sized so
    Pool (consuming TS output at 640ns/tile against DVE's 260ns/tile
    production) never starves; both streams taper to 1-tile final chunks.
  * out-DMAs: one multi-tile DMA per chunk (regular strided AP over
    out[b*256 + t*128 + p, d]) on the SP/ACT HW-DGE rings, assigned by
    exhaustive search over statically-estimated chunk completion times
    (_ring_plan) so the in-order rings never head-of-line block; one late
    job may flush on Pool's SWDGE ring (free after its last TT, longer
    ~1883ns completion latency but it unloads the HW-DGE tail).
  * input loads split across both rings (masks then the two pe halves on
    SP, c-vectors on ACT) so the first corr op starts at the per-DMA
    latency floor (~2.4us) and pe0 lands before the first combine.

Cost-model time 18.66us/core (f32 STT baseline: 33.2us).  Breakdown: head
2.42us (200ns start barrier + 1717ns DMA fixed latency + 500ns mask-DMA
floor), streams ~13.5us (the corr/add assignment is the LP optimum for
DVE TS 260 / DVE TT 400 / Pool TT 640 with Pool gated on pe0 at ~3.0us),
tail ~2.7us: the three ring-drain chains (SP/ACT + 1717, SWDGE + 1883)
end balanced within 70ns of each other, each ring saturated with the
final small chunks' flushes -- the remaining time is the streams' length.
Rejected: ACT offload (2208ns/op access-cycle init), PE rank-1 corr into
PSUM (any PSUM operand drops DVE to 1x: 925ns), copy_predicated select
(no fast mode), HBM-side scatter/accum or prefill tricks (double-write
real HBM traffic), Pool scalar_tensor_tensor single-pass (sims ~16.4us
and this axon toolchain accepts it, but the real compiler rejects TSP on
Pool), and DRAM-first-dim AP reshaping that shrinks only modeled DMA
busy (unphysical -- real bytes are unchanged).
"""

import contextlib
import os
import sys

import numpy as np

try:
    from concourse import bass, mybir
    from concourse.bass_utils import run_bass_kernel_spmd
except ImportError:
    for _p in ("/opt/trn_rl_repo", "/root/.axon_site/_ro/trn_rl_repo"):
        if os.path.isdir(_p) and _p not in sys.path:
            sys.path.insert(0, _p)
            break
    from concourse import bass, mybir
    from concourse.bass_utils import run_bass_kernel_spmd

N_CORES = 8
B, SEQ, D = 16, 2048, 768
S_SH = SEQ // N_CORES        # 256 sequence positions per core
P = 128                      # partitions
T_TILES = S_SH // P          # 2 seq tiles per core
J = B * T_TILES              # 32 output tiles per core

_F16 = mybir.dt.float16
_F32 = mybir.dt.float32

# Combine-stage work split: the fat TT chunks per seq-tile t, with an
# owner engine.  Pool (GPSIMD) adds at 640ns/tile in parallel with DVE
# (400ns/tile + init), so 20 tiles go to Pool and 12 stay on DVE; both
# streams then run ~13us.  All corr TS ops stay on DVE (TensorScalarPtr
# is walrus-rejected on Pool).  The per-t segment pattern interleaves
# pool-TS groups with dve TS+TT work, sized so the Pool stream (which
# consumes 640ns/tile against DVE's 260ns/tile TS production) never
# starves while dve-chunk completions spread across the run -- the
# chunk order below is also the out-DMA issue order per ring, keeping
# the in-order rings free of head-of-line blocking.
# Global segment order (owner, t, nb): cross-t interleaving keeps the Pool
# stream fed across the t boundary; both streams taper to 1-tile final
# chunks so the last TT -> last DMA tail is short.
SEGMENTS = (
    ("pool", 0, 2), ("pool", 0, 4), ("dve", 0, 2), ("pool", 0, 4),
    ("dve", 0, 2), ("pool", 1, 4), ("pool", 1, 2), ("dve", 0, 2),
    ("dve", 1, 2), ("pool", 1, 2), ("dve", 1, 2), ("pool", 1, 1),
    ("dve", 1, 1), ("pool", 1, 1), ("dve", 1, 1),
)

_prog_cache: dict = {}
LAST_RESULTS = None          # BassKernelResults of the most recent run


def _chunk_list():
    """[(owner, t, b0, nb), ...] in TS-emission order; per t, pool owns
    the low b range, dve the high b."""
    out = []
    bp = [0] * T_TILES
    npool = [sum(nb for o, t_, nb in SEGMENTS if o == "pool" and t_ == t)
             for t in range(T_TILES)]
    bd = list(npool)
    for own, t, nb in SEGMENTS:
        if own == "pool":
            out.append((own, t, bp[t], nb))
            bp[t] += nb
        else:
            out.append((own, t, bd[t], nb))
            bd[t] += nb
    for t in range(T_TILES):
        assert bd[t] == B and bp[t] == npool[t]
    return out


def _split_last(chunks):
    """Column-split chunk set -- measured counterproductive (extra TT init
    and sem hops outweigh the ring parallelism), so disabled."""
    return set()


def _job_list(chunks, split):
    """DMA jobs [(chunk_idx, owner, t, b0, nb, c0, c1, seq)] where seq is
    the owner's TT-retire count the job's DMA waits for."""
    jobs = []
    cnt = {"dve": 0, "pool": 0}
    for k, (own, t, b0, nb) in enumerate(chunks):
        if k in split:
            for c0, c1 in ((0, D // 2), (D // 2, D)):
                cnt[own] += 1
                jobs.append((k, own, t, b0, nb, c0, c1, cnt[own]))
        else:
            cnt[own] += 1
            jobs.append((k, own, t, b0, nb, 0, D, cnt[own]))
    return jobs


def _ring_plan(chunks, tok_cols):
    """Static completion-time estimate per DMA job (cost-model
    arithmetic), then exhaustive assignment of the jobs to the two HW-DGE
    rings (order within a ring = ready order).  Returns {ring: [job,...]}."""
    TS_N, TT_DVE, TT_DVE_INIT, TT_POOL, STT_N = 260.0, 400.0, 60.0, 640.0, 860.0
    SEM = 35.0
    t_head = 2417.0                  # masks-DMA completion gates first TS
    pe_done = (3009.0, 3601.0)       # pe0/pe1 completion (SP ring queue)
    split = _split_last(chunks)
    jobs = _job_list(chunks, split)
    dve = t_head
    pool = 0.0
    ts_done = {}
    ready = {}                       # job index -> est completion
    jd = {k: [j for j, jj in enumerate(jobs) if jj[0] == k] for k in range(len(chunks))}
    for k, (own, t, b0, nb) in enumerate(chunks):
        for b in range(b0, b0 + nb):
            dve += TS_N
            if b * T_TILES + t in tok_cols:
                dve += STT_N
        ts_done[k] = dve + SEM
        if own == "dve":
            dve = max(dve, pe_done[t])
            for j in jd[k]:
                c0, c1 = jobs[j][5], jobs[j][6]
                dve += (c1 - c0) * TT_DVE / D * nb + TT_DVE_INIT
                ready[j] = dve + SEM
    for k, (own, t, b0, nb) in enumerate(chunks):
        if own != "pool":
            continue
        pool = max(pool, ts_done[k], pe_done[t])
        for j in jd[k]:
            c0, c1 = jobs[j][5], jobs[j][6]
            pool += (c1 - c0) * TT_POOL / D * nb
            ready[j] = pool + SEM
    order = sorted(range(len(jobs)), key=lambda j: ready[j])
    busy = [max(500.0, jobs[j][4] * (jobs[j][6] - jobs[j][5]) * 2 * 0.3855)
            for j in range(len(jobs))]
    n = len(jobs)

    def pack2(idxs):
        """Exhaustive 2-ring assignment of jobs `idxs` (in ready order);
        returns (makespan, rings)."""
        best = None
        for bits in range(1 << len(idxs)):
            free = [0.0, 0.0]
            for pos, j in enumerate(idxs):
                r = (bits >> pos) & 1
                free[r] = max(free[r], ready[j]) + busy[j]
            m = max(free)
            if best is None or m < best[0]:
                best = (m, bits)
        rings = {0: [], 1: []}
        for pos, j in enumerate(idxs):
            rings[(best[1] >> pos) & 1].append(jobs[j])
        return best[0], rings

    if n > 18:
        rings = {0: [], 1: [], 2: []}
        free = {0: 0.0, 1: 0.0}
        for j in order:
            r = min((0, 1), key=lambda r: max(free[r], ready[j]) + busy[j])
            free[r] = max(free[r], ready[j]) + busy[j]
            rings[r].append(jobs[j])
        return rings

    # Option A: everything on the SP/ACT rings.
    mk2, rings2 = pack2(order)
    best = (mk2 + 1716.7, {0: rings2[0], 1: rings2[1], 2: []})
    # Option B: one late job flushes via Pool's SWDGE ring, which is free
    # after the last pool TT.  SWDGE completion latency is longer
    # (~1883ns vs 1717ns) so only worth it when it unloads the makespan.
    pool_free = max((ready[j] for j in range(n) if jobs[j][1] == "pool"),
                    default=0.0)
    import itertools
    for nr2 in (1, 2):
        for cands in itertools.combinations(order[-4:], nr2):
            cl = sorted(cands, key=lambda j: ready[j])
            rest = [j for j in order if j not in cands]
            mk2, rings2 = pack2(rest)
            f2 = pool_free
            for j in cl:
                f2 = max(f2, ready[j]) + busy[j]
            m = max(mk2 + 1716.7, f2 + 1883.3)
            if m < best[0]:
                best = (m, {0: rings2[0], 1: rings2[1],
                            2: [jobs[j] for j in cl]})
    return best[1]


def _build(tok_cols: frozenset) -> "bass.Bass":
    need_tok = bool(tok_cols)
    # consts fp16: [cseg 768 | (ctok 768) | pe0 768 | pe1 768]
    CSEG = 0
    CTOK = D if need_tok else None
    PE = (2 * D) if need_tok else D
    C = PE + T_TILES * D
    NM = 2 * J if need_tok else J      # fp32 mask cols: m2 | (m1)

    nc = bass.Bass("TRN2")
    const_d = nc.dram_tensor("consts", [P, C], _F16, kind="ExternalInput")
    mask_d = nc.dram_tensor("masks", [P, NM], _F32, kind="ExternalInput")
    out_d = nc.dram_tensor("out", [B * S_SH, D], _F16, kind="ExternalOutput")
    mult, add = mybir.AluOpType.mult, mybir.AluOpType.add

    chunks = _chunk_list()

    with contextlib.ExitStack() as stack:
        c_t = stack.enter_context(nc.sbuf_tensor([P, C], _F16))
        m_t = stack.enter_context(nc.sbuf_tensor([P, NM], _F32))
        # slot s = t*16 + b; slots of one t contiguous so a chunk is one AP
        slots = stack.enter_context(nc.sbuf_tensor([P, J * D], _F16))
        m_sem = stack.enter_context(nc.semaphore("m_sem"))
        c_sem = stack.enter_context(nc.semaphore("c_sem"))
        b_sems = [stack.enter_context(nc.semaphore(f"b{t}_sem"))
                  for t in range(T_TILES)]
        v_sem = stack.enter_context(nc.semaphore("v_sem"))
        p_sem = stack.enter_context(nc.semaphore("p_sem"))
        o_sem = stack.enter_context(nc.semaphore("o_sem"))
        po_sem = stack.enter_context(nc.semaphore("po_sem"))
        # DVE pipeline is deep: ops reading a same-engine predecessor's
        # output need an explicit retire guard; ts_sem counts corr-op
        # retires and also carries the cross-engine TS->PoolTT ordering.
        ts_sem = stack.enter_context(nc.semaphore("ts_sem"))
        block = stack.enter_context(nc.Block())

        cseg = c_t[:, CSEG:CSEG + D]
        ctok = c_t[:, CTOK:CTOK + D] if need_tok else None

        # corr-op retire count after each chunk's TS group (filled while
        # emitting the vector block, read by pool/DMA emitters)
        ts_after: dict = {}
        split = _split_last(chunks)

        def grp_ap(t, b0, nb, c0=0, c1=D):
            s0 = t * B + b0
            ap = slots[:, s0 * D:(s0 + nb) * D].rearrange(
                "p (s d) -> p s d", s=nb)
            return ap[:, :, c0:c1] if (c0, c1) != (0, D) else ap

        def pe_bcast(t, nb, c0=0, c1=D):
            return c_t[:, PE + t * D + c0:PE + t * D + c1].unsqueeze(1) \
                .broadcast_to([P, nb, c1 - c0])

        def tt_parts(k):
            """Column ranges of chunk k's TT instruction(s)."""
            if k in split:
                return ((0, D // 2), (D // 2, D))
            return ((0, D),)

        ring_plan = _ring_plan(chunks, tok_cols)

        def issue_out(eng, ring):
            # out-DMA per job waits its owner's TT-retire count;
            # dst element (p, b, d) -> row b*256 + t*128 + p, col d
            out_v = out_d[:, :].rearrange(
                "(b t p) d -> p t b d", b=B, t=T_TILES)
            for (k, own, t, b0, nb, c0, c1, seq) in ring_plan[ring]:
                done = (v_sem if own == "dve" else p_sem, seq)
                # SWDGE-updated semaphores must start from 0 -- the pool
                # ring gets its own completion sem
                osem = po_sem if ring == 2 else o_sem
                eng.dma_start(out_v[:, t, b0:b0 + nb, c0:c1],
                              grp_ap(t, b0, nb, c0, c1)) \
                    ._wait_ge(*done).then_inc(osem, 16)

        @block.sync
        def _(sync):
            # SP ring: masks (tiny, gates the first corr op), then the two
            # pe halves -- pe0 lands early so the TT streams start sooner
            sync.dma_start(m_t[:], mask_d[:]).then_inc(m_sem, 16)
            for t in range(T_TILES):
                sync.dma_start(c_t[:, PE + t * D:PE + (t + 1) * D],
                               const_d[:, PE + t * D:PE + (t + 1) * D]) \
                    .then_inc(b_sems[t], 16)
            issue_out(sync, 0)

        @block.scalar
        def _(scalar):
            # ACT ring: c-vectors, in parallel with the SP loads
            scalar.dma_start(c_t[:, :PE], const_d[:, :PE]).then_inc(c_sem, 16)
            issue_out(scalar, 1)

        @block.vector
        def _(vector):
            nc.vector.wait_ge(m_sem, 16)
            nc.vector.wait_ge(c_sem, 16)
            waited_b = [False] * T_TILES
            ncorr = 0
            for k, (own, t, b0, nb) in enumerate(chunks):
                for b in range(b0, b0 + nb):
                    j = b * T_TILES + t      # mask column index
                    s = t * B + b
                    sl = slots[:, s * D:(s + 1) * D]
                    nc.vector.tensor_scalar(
                        sl, cseg, scalar1=m_t[:, j:j + 1], op0=mult,
                        scalar2=None).then_inc(ts_sem, 1)
                    ncorr += 1
                    if j in tok_cols:
                        nc.vector.scalar_tensor_tensor(
                            sl, ctok, m_t[:, J + j:J + j + 1], sl,
                            op0=mult, op1=add,
                        )._wait_ge(ts_sem, ncorr).then_inc(ts_sem, 1)
                        ncorr += 1
                ts_after[k] = ncorr
                if own != "dve":
                    continue
                if not waited_b[t]:
                    nc.vector.wait_ge(b_sems[t], 16)
                    waited_b[t] = True
                for c0, c1 in tt_parts(k):
                    nc.vector.tensor_tensor(
                        grp_ap(t, b0, nb, c0, c1), grp_ap(t, b0, nb, c0, c1),
                        pe_bcast(t, nb, c0, c1),
                        op=add)._wait_ge(ts_sem, ncorr).then_inc(v_sem, 1)

        @block.gpsimd
        def _(gpsimd):
            waited_b = [False] * T_TILES
            for k, (own, t, b0, nb) in enumerate(chunks):
                if own != "pool":
                    continue
                if not waited_b[t]:
                    gpsimd.wait_ge(b_sems[t], 16)
                    waited_b[t] = True
                for c0, c1 in tt_parts(k):
                    nc.gpsimd.tensor_tensor(
                        grp_ap(t, b0, nb, c0, c1), grp_ap(t, b0, nb, c0, c1),
                        pe_bcast(t, nb, c0, c1),
                        op=add)._wait_ge(ts_sem, ts_after[k]).then_inc(p_sem, 1)
            # late-job flush on the SWDGE ring once the pool TT stream ends
            issue_out(gpsimd, 2)

    nc.finalize()
    return nc


def _prepare(inputs: dict):
    ids = np.asarray(inputs["input_ids"])
    seg = np.asarray(inputs["segment_label"])
    W_tok = np.asarray(inputs["W_tok"], dtype=np.float32)
    b_tok = np.asarray(inputs["b_tok"], dtype=np.float32)
    W_seg = np.asarray(inputs["W_seg"], dtype=np.float32)
    b_seg = np.asarray(inputs["b_seg"], dtype=np.float32)
    pe = np.asarray(inputs["pe"], dtype=np.float32).reshape(SEQ, D)

    c_tok = (W_tok[:, 0] + b_tok).astype(np.float16)
    c_seg = (W_seg[:, 0] + b_seg).astype(np.float16)
    m1_full = (ids == 0).astype(np.float32)
    m2_full = (seg == 0).astype(np.float32)

    per_core = []
    tok_cols = set()
    for c in range(N_CORES):
        sl = slice(c * S_SH, (c + 1) * S_SH)
        # [B, S_SH] -> [P, J] with column j = b*T_TILES + t, partition p
        m1 = m1_full[:, sl].reshape(B, T_TILES, P).transpose(2, 0, 1).reshape(P, J)
        m2 = m2_full[:, sl].reshape(B, T_TILES, P).transpose(2, 0, 1).reshape(P, J)
        pe_sl = pe[sl].reshape(T_TILES, P, D).transpose(1, 0, 2) \
            .reshape(P, T_TILES * D).astype(np.float16)
        tok_cols.update(np.nonzero(m1.any(axis=0))[0].tolist())
        per_core.append((pe_sl, m1, m2))

    # SPMD: one program for all cores; a tile gets the tok STT if any core
    # needs it (a zero mask column makes it the identity elsewhere).
    tok_cols = frozenset(tok_cols)
    need_tok = bool(tok_cols)
    CSEG = 0
    CTOK = D if need_tok else None
    PE = (2 * D) if need_tok else D
    C = PE + T_TILES * D
    NM = 2 * J if need_tok else J

    in_maps = []
    for pe_sl, m1, m2 in per_core:
        consts = np.empty((P, C), dtype=np.float16)
        consts[:, CSEG:CSEG + D] = c_seg
        if need_tok:
            consts[:, CTOK:CTOK + D] = c_tok
        consts[:, PE:] = pe_sl
        masks = np.empty((P, NM), dtype=np.float32)
        masks[:, :J] = m2
        if need_tok:
            masks[:, J:] = m1
        in_maps.append({"consts": consts, "masks": masks})
    return in_maps, tok_cols


def kernel(**inputs) -> np.ndarray:
    global LAST_RESULTS
    in_maps, tok_cols = _prepare(inputs)
    key = ("v3", SEGMENTS, tok_cols)
    if key not in _prog_cache:
        _prog_cache[key] = _build(tok_cols)
    nc = _prog_cache[key]

    trace = bool(int(os.environ.get("BASS_KERNEL_TRACE", "0")))
    try:
        res = run_bass_kernel_spmd(
            nc, in_maps, list(range(N_CORES)), trace=trace,
            trace_cores=list(range(N_CORES)) if trace else None,
        )
    except ModuleNotFoundError:
        # axon builds without the NTFF profile hook (antenv.axon_hooks)
        # crash when tracing is requested (e.g. BASS_TRACE=1 in the env);
        # degrade to an untraced run rather than failing the kernel.
        os.environ["BASS_NEVER_TRACE"] = "1"
        res = run_bass_kernel_spmd(nc, in_maps, list(range(N_CORES)), trace=False)
    LAST_RESULTS = res

    out = np.empty((B, SEQ, D), dtype=np.float32)
    for c in range(N_CORES):
        out[:, c * S_SH:(c + 1) * S_SH, :] = (
            np.asarray(res.results[c]["out"]).astype(np.float32)
            .reshape(B, S_SH, D)
        )
    return out


# revision 35
# speedup vs baseline: 1.7971x; 1.0089x over previous
"""Trainium2 Bass kernel for nn_BeBertEmbedding (self-contained).

Math: the reference's semantic_embed(ids, W, b, pad=0) is
    where(ids==0, take(W.T, ids) + b, zeros)
so the only table row that survives is W[:, 0], and the whole module is
    out[b,s,:] = pe[s,:] + (ids[b,s]==0)*(W_tok[:,0]+b_tok)
                         + (seg[b,s]==0)*(W_seg[:,0]+b_seg)

Sharding: sequence-parallel across 8 cores (256 positions/core, all 16
batches; each core writes a disjoint [16, 256, 768] slice, no collectives).

Everything on device is fp16 (tolerance is rel 2e-2; fp16 keeps us ~7e-4):
halves HBM traffic vs f32 and unlocks the DVE 16-bit perf modes.  The host
upcasts the returned fp16 shard to float32.

Per core the program is raw Bass, 32 output tiles [128 tokens, 768] resident
in SBUF at once (49KB/partition, no slot recycling):
  * corr stage (DVE): one tensor_scalar per tile (cseg * m2[p]) -- fp16
    data with an fp32 per-partition mask scalar runs in the 4x DVE mode
    (260ns vs 860ns for the fused scalar_tensor_tensor, which has no fast
    mode); the rare tiles with a zero token id get one extra STT.
  * combine stage (DVE + Pool in parallel): per chunk of 1-4 tiles one fat
    tensor_tensor adds the seq-tile's pe broadcast across the chunk
    (stride-0 mid-dim AP).  20 tiles go to the otherwise-idle GPSIMD at
    640ns/tile (the documented 153 Gelem/s rate; plain TensorTensor only
    -- TensorScalarPtr on Pool is walrus-rejected, NCC_IXCG966) and 12
    stay on DVE at 2x mode (~400ns/tile); with DVE also carrying all 32
    corr ops, both streams run ~13us and finish together.  SEGMENTS
    interleaves pool-TS groups with dve TS+TT work, cross-t, sized so
    Pool (consuming TS output at 640ns/tile against DVE's 260ns/tile
    production) never starves; both streams taper to 1-tile final chunks.
  * out-DMAs: one multi-tile DMA per chunk (regular strided AP over
    out[b*256 + t*128 + p, d]) on the SP/ACT HW-DGE rings, assigned by
    exhaustive search over statically-estimated chunk completion times
    (_ring_plan) so the in-order rings never head-of-line block; one late
    job may flush on Pool's SWDGE ring (free after its last TT, longer
    ~1883ns completion latency but it unloads the HW-DGE tail).
  * input loads on all three rings: masks on SP (they gate the first corr
    op, so they ride alone at the per-DMA latency floor), c-vectors on
    ACT, and the two pe halves SELF-LOADED on Pool's SWDGE ring -- Pool
    would idle waiting for them anyway, and first-in-queue they complete
    ~330ns earlier, letting Pool's combine stream start at ~2.8us (gated
    by its first 1-tile chunk's corr, not by pe).

Cost-model time 18.49us/core (f32 STT baseline: 33.2us).  Breakdown: head
2.42us (200ns start barrier + 1717ns DMA fixed latency + 500ns mask-DMA
floor), then the end is exactly DVE-stream-end (15957 = head + 32x260 TS
+ 12-tile TT share, the LP optimum against Pool TT 640) plus the minimal
flush chain (sem 25 + DMA 592 + 1717 latency + 240 drains/barrier); the
ACT and SWDGE drain chains carry ~230ns of slack, Pool ends at 15.58us
with no feed stalls (segment order found by sim-in-the-loop search).
Rejected: ACT offload (2208ns/op access-cycle init), PE rank-1 corr into
PSUM (any PSUM operand drops DVE to 1x: 925ns), copy_predicated select
(no fast mode), HBM-side scatter/accum or prefill tricks (double-write
real HBM traffic), Pool scalar_tensor_tensor single-pass (sims ~16.4us
and this axon toolchain accepts it, but the real compiler rejects TSP on
Pool), and DRAM-first-dim AP reshaping that shrinks only modeled DMA
busy (unphysical -- real bytes are unchanged).
"""

import contextlib
import os
import sys

import numpy as np

try:
    from concourse import bass, mybir
    from concourse.bass_utils import run_bass_kernel_spmd
except ImportError:
    for _p in ("/opt/trn_rl_repo", "/root/.axon_site/_ro/trn_rl_repo"):
        if os.path.isdir(_p) and _p not in sys.path:
            sys.path.insert(0, _p)
            break
    from concourse import bass, mybir
    from concourse.bass_utils import run_bass_kernel_spmd

N_CORES = 8
B, SEQ, D = 16, 2048, 768
S_SH = SEQ // N_CORES        # 256 sequence positions per core
P = 128                      # partitions
T_TILES = S_SH // P          # 2 seq tiles per core
J = B * T_TILES              # 32 output tiles per core

_F16 = mybir.dt.float16
_F32 = mybir.dt.float32

# Combine-stage work split: the fat TT chunks per seq-tile t, with an
# owner engine.  Pool (GPSIMD) adds at 640ns/tile in parallel with DVE
# (400ns/tile + init), so 20 tiles go to Pool and 12 stay on DVE; both
# streams then run ~13us.  All corr TS ops stay on DVE (TensorScalarPtr
# is walrus-rejected on Pool).  The per-t segment pattern interleaves
# pool-TS groups with dve TS+TT work, sized so the Pool stream (which
# consumes 640ns/tile against DVE's 260ns/tile TS production) never
# starves while dve-chunk completions spread across the run -- the
# chunk order below is also the out-DMA issue order per ring, keeping
# the in-order rings free of head-of-line blocking.
# Global segment order (owner, t, nb): cross-t interleaving keeps the Pool
# stream fed across the t boundary; both streams taper to 1-tile final
# chunks so the last TT -> last DMA tail is short.
SEGMENTS = (
    ("pool", 0, 1), ("pool", 0, 1), ("pool", 0, 4), ("dve", 0, 2),
    ("pool", 0, 4), ("dve", 0, 2), ("pool", 1, 4), ("pool", 1, 2),
    ("dve", 0, 2), ("dve", 1, 2), ("pool", 1, 2), ("dve", 1, 2),
    ("pool", 1, 1), ("pool", 1, 1), ("dve", 1, 1), ("dve", 1, 1),
)

_prog_cache: dict = {}
LAST_RESULTS = None          # BassKernelResults of the most recent run


def _chunk_list():
    """[(owner, t, b0, nb), ...] in TS-emission order; per t, pool owns
    the low b range, dve the high b."""
    out = []
    bp = [0] * T_TILES
    npool = [sum(nb for o, t_, nb in SEGMENTS if o == "pool" and t_ == t)
             for t in range(T_TILES)]
    bd = list(npool)
    for own, t, nb in SEGMENTS:
        if own == "pool":
            out.append((own, t, bp[t], nb))
            bp[t] += nb
        else:
            out.append((own, t, bd[t], nb))
            bd[t] += nb
    for t in range(T_TILES):
        assert bd[t] == B and bp[t] == npool[t]
    return out


def _split_last(chunks):
    """Column-split chunk set -- measured counterproductive (extra TT init
    and sem hops outweigh the ring parallelism), so disabled."""
    return set()


# The last 1-tile dve chunk's combine can be split across engines: DVE
# adds cols [0, MIXC), Pool adds [MIXC, D) as its final TT.  Measured
# counterproductive at every MIXC (pool's late stream + ring replan eat
# the DVE saving), so disabled; the machinery stays for reference.
MIXC = 0


def _mix_chunk(chunks):
    ks = [k for k, c in enumerate(chunks) if c[0] == "dve" and c[3] == 1]
    return ks[-1] if (MIXC and ks) else None


def _job_list(chunks, split):
    """DMA jobs [(chunk_idx, owner, t, b0, nb, c0, c1, seq)] where seq is
    the owner's TT-retire count the job's DMA waits for."""
    km = _mix_chunk(chunks)
    jobs = []
    cnt = {"dve": 0, "pool": 0}
    for k, (own, t, b0, nb) in enumerate(chunks):
        if k == km:
            cnt["dve"] += 1
            jobs.append((k, "dve", t, b0, nb, 0, MIXC, cnt["dve"]))
        elif k in split:
            for c0, c1 in ((0, D // 2), (D // 2, D)):
                cnt[own] += 1
                jobs.append((k, own, t, b0, nb, c0, c1, cnt[own]))
        else:
            cnt[own] += 1
            jobs.append((k, own, t, b0, nb, 0, D, cnt[own]))
    if km is not None:
        # pool's very last TT is the mix half
        _, t, b0, nb = chunks[km]
        jobs.append((km, "pool", t, b0, nb, MIXC, D, cnt["pool"] + 1))
    return jobs


def _ring_plan(chunks, tok_cols):
    """Static completion-time estimate per DMA job (cost-model
    arithmetic), then exhaustive assignment of the jobs to the two HW-DGE
    rings (order within a ring = ready order).  Returns {ring: [job,...]}."""
    TS_N, TT_DVE, TT_DVE_INIT, TT_POOL, STT_N = 260.0, 400.0, 60.0, 640.0, 860.0
    SEM = 35.0
    t_head = 2417.0                  # masks-DMA completion gates first TS
    # pe halves self-load via SWDGE (issue 200, +1883 latency, 592 busy)
    pe_done = (2675.0, 3267.0)
    split = _split_last(chunks)
    jobs = _job_list(chunks, split)
    dve = t_head
    pool = 1384.0                    # pool engine busy with the pe loads
    ts_done = {}
    ready = {}                       # job index -> est completion
    jd = {k: [j for j, jj in enumerate(jobs) if jj[0] == k] for k in range(len(chunks))}
    for k, (own, t, b0, nb) in enumerate(chunks):
        for b in range(b0, b0 + nb):
            dve += TS_N
            if b * T_TILES + t in tok_cols:
                dve += STT_N
        ts_done[k] = dve + SEM
        if own == "dve":
            dve = max(dve, pe_done[t])
            for j in jd[k]:
                c0, c1 = jobs[j][5], jobs[j][6]
                dve += (c1 - c0) * TT_DVE / D * nb + TT_DVE_INIT
                ready[j] = dve + SEM
    km = _mix_chunk(chunks)
    for k, (own, t, b0, nb) in enumerate(chunks):
        if own != "pool":
            continue
        pool = max(pool, ts_done[k], pe_done[t])
        for j in jd[k]:
            if jobs[j][1] != "pool":
                continue
            c0, c1 = jobs[j][5], jobs[j][6]
            pool += (c1 - c0) * TT_POOL / D * nb
            ready[j] = pool + SEM
    if km is not None:
        # pool's final TT: the mix chunk's high columns
        jmix = [j for j in jd[km] if jobs[j][1] == "pool"][0]
        pool = max(pool, ts_done[km], pe_done[chunks[km][1]])
        pool += (D - MIXC) * TT_POOL / D
        ready[jmix] = pool + SEM
    order = sorted(range(len(jobs)), key=lambda j: ready[j])
    busy = [max(500.0, jobs[j][4] * (jobs[j][6] - jobs[j][5]) * 2 * 0.3855)
            for j in range(len(jobs))]
    n = len(jobs)

    def pack2(idxs):
        """Exhaustive 2-ring assignment of jobs `idxs` (in ready order);
        returns (makespan, rings)."""
        best = None
        for bits in range(1 << len(idxs)):
            free = [0.0, 0.0]
            for pos, j in enumerate(idxs):
                r = (bits >> pos) & 1
                free[r] = max(free[r], ready[j]) + busy[j]
            m = max(free)
            if best is None or m < best[0]:
                best = (m, bits)
        rings = {0: [], 1: []}
        for pos, j in enumerate(idxs):
            rings[(best[1] >> pos) & 1].append(jobs[j])
        return best[0], rings

    if n > 18:
        rings = {0: [], 1: [], 2: []}
        free = {0: 0.0, 1: 0.0}
        for j in order:
            r = min((0, 1), key=lambda r: max(free[r], ready[j]) + busy[j])
            free[r] = max(free[r], ready[j]) + busy[j]
            rings[r].append(jobs[j])
        return rings

    # Option A: everything on the SP/ACT rings.
    mk2, rings2 = pack2(order)
    best = (mk2 + 1716.7, {0: rings2[0], 1: rings2[1], 2: []})
    # Option B: one late job flushes via Pool's SWDGE ring, which is free
    # after the last pool TT.  SWDGE completion latency is longer
    # (~1883ns vs 1717ns) so only worth it when it unloads the makespan.
    pool_free = max((ready[j] for j in range(n) if jobs[j][1] == "pool"),
                    default=0.0)
    import itertools
    for nr2 in (1, 2):
        for cands in itertools.combinations(order[-4:], nr2):
            cl = sorted(cands, key=lambda j: ready[j])
            rest = [j for j in order if j not in cands]
            mk2, rings2 = pack2(rest)
            f2 = pool_free
            for j in cl:
                f2 = max(f2, ready[j]) + busy[j]
            m = max(mk2 + 1716.7, f2 + 1883.3)
            if m < best[0]:
                best = (m, {0: rings2[0], 1: rings2[1],
                            2: [jobs[j] for j in cl]})
    return best[1]


def _build(tok_cols: frozenset) -> "bass.Bass":
    need_tok = bool(tok_cols)
    # consts fp16: [cseg 768 | (ctok 768) | pe0 768 | pe1 768]
    CSEG = 0
    CTOK = D if need_tok else None
    PE = (2 * D) if need_tok else D
    C = PE + T_TILES * D
    NM = 2 * J if need_tok else J      # fp32 mask cols: m2 | (m1)

    nc = bass.Bass("TRN2")
    const_d = nc.dram_tensor("consts", [P, C], _F16, kind="ExternalInput")
    mask_d = nc.dram_tensor("masks", [P, NM], _F32, kind="ExternalInput")
    out_d = nc.dram_tensor("out", [B * S_SH, D], _F16, kind="ExternalOutput")
    mult, add = mybir.AluOpType.mult, mybir.AluOpType.add

    chunks = _chunk_list()

    with contextlib.ExitStack() as stack:
        c_t = stack.enter_context(nc.sbuf_tensor([P, C], _F16))
        m_t = stack.enter_context(nc.sbuf_tensor([P, NM], _F32))
        # slot s = t*16 + b; slots of one t contiguous so a chunk is one AP
        slots = stack.enter_context(nc.sbuf_tensor([P, J * D], _F16))
        m_sem = stack.enter_context(nc.semaphore("m_sem"))
        c_sem = stack.enter_context(nc.semaphore("c_sem"))
        b_sems = [stack.enter_context(nc.semaphore(f"b{t}_sem"))
                  for t in range(T_TILES)]
        v_sem = stack.enter_context(nc.semaphore("v_sem"))
        p_sem = stack.enter_context(nc.semaphore("p_sem"))
        o_sem = stack.enter_context(nc.semaphore("o_sem"))
        po_sem = stack.enter_context(nc.semaphore("po_sem"))
        # DVE pipeline is deep: ops reading a same-engine predecessor's
        # output need an explicit retire guard; ts_sem counts corr-op
        # retires and also carries the cross-engine TS->PoolTT ordering.
        ts_sem = stack.enter_context(nc.semaphore("ts_sem"))
        block = stack.enter_context(nc.Block())

        cseg = c_t[:, CSEG:CSEG + D]
        ctok = c_t[:, CTOK:CTOK + D] if need_tok else None

        # corr-op retire count after each chunk's TS group (filled while
        # emitting the vector block, read by pool/DMA emitters)
        ts_after: dict = {}
        split = _split_last(chunks)

        def grp_ap(t, b0, nb, c0=0, c1=D):
            s0 = t * B + b0
            ap = slots[:, s0 * D:(s0 + nb) * D].rearrange(
                "p (s d) -> p s d", s=nb)
            return ap[:, :, c0:c1] if (c0, c1) != (0, D) else ap

        def pe_bcast(t, nb, c0=0, c1=D):
            return c_t[:, PE + t * D + c0:PE + t * D + c1].unsqueeze(1) \
                .broadcast_to([P, nb, c1 - c0])

        km = _mix_chunk(chunks)

        def tt_parts(k):
            """Column ranges of chunk k's TT instruction(s) on its owner."""
            if k == km:
                return ((0, MIXC),)
            if k in split:
                return ((0, D // 2), (D // 2, D))
            return ((0, D),)

        ring_plan = _ring_plan(chunks, tok_cols)

        def issue_out(eng, ring):
            # out-DMA per job waits its owner's TT-retire count;
            # dst element (p, b, d) -> row b*256 + t*128 + p, col d
            out_v = out_d[:, :].rearrange(
                "(b t p) d -> p t b d", b=B, t=T_TILES)
            for (k, own, t, b0, nb, c0, c1, seq) in ring_plan[ring]:
                done = (v_sem if own == "dve" else p_sem, seq)
                # SWDGE-updated semaphores must start from 0 -- the pool
                # ring gets its own completion sem
                osem = po_sem if ring == 2 else o_sem
                eng.dma_start(out_v[:, t, b0:b0 + nb, c0:c1],
                              grp_ap(t, b0, nb, c0, c1)) \
                    ._wait_ge(*done).then_inc(osem, 16)

        @block.sync
        def _(sync):
            # SP ring: masks only (they gate the first corr op; the pe
            # halves self-load on Pool's SWDGE ring, completing ~330ns
            # earlier than as second-in-queue here)
            sync.dma_start(m_t[:], mask_d[:]).then_inc(m_sem, 16)
            issue_out(sync, 0)

        @block.scalar
        def _(scalar):
            # ACT ring: c-vectors, in parallel with the SP loads
            scalar.dma_start(c_t[:, :PE], const_d[:, :PE]).then_inc(c_sem, 16)
            issue_out(scalar, 1)

        @block.vector
        def _(vector):
            nc.vector.wait_ge(m_sem, 16)
            nc.vector.wait_ge(c_sem, 16)
            waited_b = [False] * T_TILES
            ncorr = 0
            for k, (own, t, b0, nb) in enumerate(chunks):
                for b in range(b0, b0 + nb):
                    j = b * T_TILES + t      # mask column index
                    s = t * B + b
                    sl = slots[:, s * D:(s + 1) * D]
                    nc.vector.tensor_scalar(
                        sl, cseg, scalar1=m_t[:, j:j + 1], op0=mult,
                        scalar2=None).then_inc(ts_sem, 1)
                    ncorr += 1
                    if j in tok_cols:
                        nc.vector.scalar_tensor_tensor(
                            sl, ctok, m_t[:, J + j:J + j + 1], sl,
                            op0=mult, op1=add,
                        )._wait_ge(ts_sem, ncorr).then_inc(ts_sem, 1)
                        ncorr += 1
                ts_after[k] = ncorr
                if own != "dve":
                    continue
                if not waited_b[t]:
                    nc.vector.wait_ge(b_sems[t], 16)
                    waited_b[t] = True
                for c0, c1 in tt_parts(k):
                    nc.vector.tensor_tensor(
                        grp_ap(t, b0, nb, c0, c1), grp_ap(t, b0, nb, c0, c1),
                        pe_bcast(t, nb, c0, c1),
                        op=add)._wait_ge(ts_sem, ncorr).then_inc(v_sem, 1)

        @block.gpsimd
        def _(gpsimd):
            # pe self-loads on the SWDGE ring while Pool would be idle
            # anyway (each b_sem gets exactly one SWDGE update: software
            # DGE requires its update semaphore to start at 0)
            for t in range(T_TILES):
                gpsimd.dma_start(c_t[:, PE + t * D:PE + (t + 1) * D],
                                 const_d[:, PE + t * D:PE + (t + 1) * D]) \
                    .then_inc(b_sems[t], 16)
            waited_b = [False] * T_TILES
            for k, (own, t, b0, nb) in enumerate(chunks):
                if own != "pool":
                    continue
                if not waited_b[t]:
                    gpsimd.wait_ge(b_sems[t], 16)
                    waited_b[t] = True
                for c0, c1 in tt_parts(k):
                    nc.gpsimd.tensor_tensor(
                        grp_ap(t, b0, nb, c0, c1), grp_ap(t, b0, nb, c0, c1),
                        pe_bcast(t, nb, c0, c1),
                        op=add)._wait_ge(ts_sem, ts_after[k]).then_inc(p_sem, 1)
            if km is not None:
                # final pool TT: the mix chunk's high columns
                _, tm, bm, nm = chunks[km]
                nc.gpsimd.tensor_tensor(
                    grp_ap(tm, bm, nm, MIXC, D), grp_ap(tm, bm, nm, MIXC, D),
                    pe_bcast(tm, nm, MIXC, D),
                    op=add)._wait_ge(ts_sem, ts_after[km]).then_inc(p_sem, 1)
            # late-job flush on the SWDGE ring once the pool TT stream ends
            issue_out(gpsimd, 2)

    nc.finalize()
    return nc


def _prepare(inputs: dict):
    ids = np.asarray(inputs["input_ids"])
    seg = np.asarray(inputs["segment_label"])
    W_tok = np.asarray(inputs["W_tok"], dtype=np.float32)
    b_tok = np.asarray(inputs["b_tok"], dtype=np.float32)
    W_seg = np.asarray(inputs["W_seg"], dtype=np.float32)
    b_seg = np.asarray(inputs["b_seg"], dtype=np.float32)
    pe = np.asarray(inputs["pe"], dtype=np.float32).reshape(SEQ, D)

    c_tok = (W_tok[:, 0] + b_tok).astype(np.float16)
    c_seg = (W_seg[:, 0] + b_seg).astype(np.float16)
    m1_full = (ids == 0).astype(np.float32)
    m2_full = (seg == 0).astype(np.float32)

    per_core = []
    tok_cols = set()
    for c in range(N_CORES):
        sl = slice(c * S_SH, (c + 1) * S_SH)
        # [B, S_SH] -> [P, J] with column j = b*T_TILES + t, partition p
        m1 = m1_full[:, sl].reshape(B, T_TILES, P).transpose(2, 0, 1).reshape(P, J)
        m2 = m2_full[:, sl].reshape(B, T_TILES, P).transpose(2, 0, 1).reshape(P, J)
        pe_sl = pe[sl].reshape(T_TILES, P, D).transpose(1, 0, 2) \
            .reshape(P, T_TILES * D).astype(np.float16)
        tok_cols.update(np.nonzero(m1.any(axis=0))[0].tolist())
        per_core.append((pe_sl, m1, m2))

    # SPMD: one program for all cores; a tile gets the tok STT if any core
    # needs it (a zero mask column makes it the identity elsewhere).
    tok_cols = frozenset(tok_cols)
    need_tok = bool(tok_cols)
    CSEG = 0
    CTOK = D if need_tok else None
    PE = (2 * D) if need_tok else D
    C = PE + T_TILES * D
    NM = 2 * J if need_tok else J

    in_maps = []
    for pe_sl, m1, m2 in per_core:
        consts = np.empty((P, C), dtype=np.float16)
        consts[:, CSEG:CSEG + D] = c_seg
        if need_tok:
            consts[:, CTOK:CTOK + D] = c_tok
        consts[:, PE:] = pe_sl
        masks = np.empty((P, NM), dtype=np.float32)
        masks[:, :J] = m2
        if need_tok:
            masks[:, J:] = m1
        in_maps.append({"consts": consts, "masks": masks})
    return in_maps, tok_cols


def kernel(**inputs) -> np.ndarray:
    global LAST_RESULTS
    in_maps, tok_cols = _prepare(inputs)
    key = ("v3", SEGMENTS, tok_cols)
    if key not in _prog_cache:
        _prog_cache[key] = _build(tok_cols)
    nc = _prog_cache[key]

    trace = bool(int(os.environ.get("BASS_KERNEL_TRACE", "0")))
    try:
        res = run_bass_kernel_spmd(
            nc, in_maps, list(range(N_CORES)), trace=trace,
            trace_cores=list(range(N_CORES)) if trace else None,
        )
    except ModuleNotFoundError:
        # axon builds without the NTFF profile hook (antenv.axon_hooks)
        # crash when tracing is requested (e.g. BASS_TRACE=1 in the env);
        # degrade to an untraced run rather than failing the kernel.
        os.environ["BASS_NEVER_TRACE"] = "1"
        res = run_bass_kernel_spmd(nc, in_maps, list(range(N_CORES)), trace=False)
    LAST_RESULTS = res

    out = np.empty((B, SEQ, D), dtype=np.float32)
    for c in range(N_CORES):
        out[:, c * S_SH:(c + 1) * S_SH, :] = (
            np.asarray(res.results[c]["out"]).astype(np.float32)
            .reshape(B, S_SH, D)
        )
    return out
